# revision 44
# baseline (speedup 1.0000x reference)
"""MoE (8 experts, top-2) Bass kernel for 8 trn2 NeuronCores.

Strategy: data-parallel over tokens with HOST-BALANCED token->core assignment.
The host groups tokens by their top-2 expert pair (type) and deals them
round-robin across the 8 cores, so every core sees ~C_e/8 tokens per expert.
That pins the per-expert capacity CAP_e at ~512, which removes the padded
5th proj sub-tile and the fc tail matmuls that dominate PE waste when
capacities are unbalanced.

On device, per core:
  phase 1: router logits via fp32 matmuls on host-transposed x chunks
           (per-k-chunk DMAs pipelined with PE warmup), batched top-2 /
           combine-weight math, counts -> offsets via an on-chip prefix
           matmul (constant prefix mask; no DRAM round-trips), positions,
           then ONE (w, token_id) record scatter into the position-ordered
           dispatch table. The scatter's wrapped-16 index layout is staged
           through DRAM with 8 small strided writes spread over all three
           DMA rings + contiguous replicated readbacks, replacing the
           baseline's 8 full strided re-reads (~70us -> ~15us).
  phase 2: per expert: dma_gather(transpose=True) routed token rows into
           [H-chunk, c] matmul layout, dense FFN gelu(x@Wfc)@Wproj in bf16
           with fp32 accumulate (fc split at 256 for over-512 capacities so
           no matmul is LDWEIGHTS-bound), scale rows by combine weight, one
           dma_scatter_add per expert. Ring discipline (DMA queues are FIFO
           and a waiting dma_start blocks its ENGINE): sync carries only the
           paired proj-weight stream + expert metadata whose waits have
           slack; scalar carries activations + fc weight prefetch (2 experts
           ahead; during phase 1 on the idle gpsimd ring); gpsimd carries
           zero-init, the wrapped tid read (element-granular), gathers and
           scatters. The wrapped gather indices are read once into 16
           partitions, cast to i16, and replicated to all 8 gpsimd-core
           groups via a tiny contiguous DRAM bounce. PE p-state is re-ramped
           with dependency-gated warmup matmuls right before expert 0.

Host does only slicing/concat/layout staging (permutation + transpose +
bf16 cast) plus a routing peek to pick compile-time capacities and the
balanced assignment.
"""

import math
import os
import sys

import numpy as np

for _p in ("/opt/trn_rl_repo", "/root/.axon_site/_ro/trn_rl_repo"):
    if os.path.isdir(_p) and _p not in sys.path:
        sys.path.insert(0, _p)

import ml_dtypes  # noqa: E402
import concourse.bass as bass  # noqa: E402
import concourse.mybir as mybir  # noqa: E402
import concourse.tile as tile  # noqa: E402
from concourse import bacc  # noqa: E402
from concourse.masks import make_upper_triangular, make_identity  # noqa: E402
from concourse import library_config  # noqa: E402

F32 = mybir.dt.float32
BF16 = mybir.dt.bfloat16
I32 = mybir.dt.int32
I16 = mybir.dt.int16
AF = mybir.ActivationFunctionType
ALU = mybir.AluOpType
AX = mybir.AxisListType

N_CORES = 8
P = 128
_ACT_FN = AF.Gelu_apprx_tanh  # debug hook: sim lacks gelu, tests swap in Tanh


def _chunks(total, step):
    out = []
    off = 0
    while off < total:
        w = min(step, total - off)
        out.append((off, w))
        off += w
    return out


def build_moe(TLOC, H, F, E, CAPS, zero_bias=False):
    """Build the per-core Bass program (SPMD: identical on all cores)."""
    assert TLOC % P == 0 and H % P == 0 and F % P == 0 and E == 8
    CAPS = list(CAPS)
    assert len(CAPS) == E and all(c % 8 == 0 for c in CAPS)
    CAPRS = [((c + P - 1) // P) * P for c in CAPS]
    KH = H // P            # contraction chunks over H (6)
    KF = F // P            # f-tiles (and stage-2 contraction chunks) (24)
    NT = TLOC // P         # token tiles (16)
    assert NT == 16
    BASES = [sum(CAPRS[:e]) for e in range(E)]
    NPOS = sum(CAPRS) + P
    NPOS = ((NPOS + P - 1) // P) * P
    NBLK = NPOS // P
    HT = ((0, 512), (512, H - 512))  # proj h-tiles

    nc = bacc.Bacc("TRN2", target_bir_lowering=False, debug=False,
                   enable_asserts=True, num_devices=N_CORES)

    xt = nc.dram_tensor("xt", [H, TLOC], F32, kind="ExternalInput")
    xbf = nc.dram_tensor("xbf", [TLOC + P, H], BF16, kind="ExternalInput")
    wr = nc.dram_tensor("wr", [H, E], F32, kind="ExternalInput")
    wfc = nc.dram_tensor("wfc", [E, H, F], BF16, kind="ExternalInput")
    wpj = nc.dram_tensor("wpj", [E, F, H], BF16, kind="ExternalInput")
    if not zero_bias:
        brr = nc.dram_tensor("brr", [1, E], F32, kind="ExternalInput")
        bfc = nc.dram_tensor("bfc", [E, F], F32, kind="ExternalInput")
        bpj = nc.dram_tensor("bpj", [E, H], BF16, kind="ExternalInput")
    # extra trash tile rows at the end absorb pad-slot scatter-adds
    out = nc.dram_tensor("out", [TLOC + P, H], F32, kind="ExternalOutput")

    # dispatch table: (combine w, token id) in cols 0:2 of 256B-strided rows
    # (dma_scatter_add needs a 256B row stride)
    wtbuf = nc.dram_tensor("wtbuf", [NPOS, 64], F32)
    # wrapped-16 dispatch index staging: PD2[p%16, 8*r + p//16] = dsi[p, r]
    NCOL = 2 * TLOC // 16
    pd2 = nc.dram_tensor("pd2", [16, NCOL], I16)
    # per-expert gather-index replication bounce (i16, wrap-16 layout)
    MAXW = max(CAPRS) // 16
    pdx = nc.dram_tensor("pdx", [E, 16, MAXW], I16)

    with tile.TileContext(nc) as tc:
        with tc.tile_pool(name="const", bufs=1) as cpool, \
             tc.tile_pool(name="wc", bufs=2) as wc:
            # ---------------- constants ----------------
            u_incl = cpool.tile([P, P], F32, tag="u_incl")
            make_upper_triangular(nc, u_incl, val=1.0, diag=True)
            ones_f = cpool.tile([1, 512], F32, tag="ones_f")
            nc.gpsimd.memset(ones_f[:], 1.0)
            ones_col = cpool.tile([P, 1], F32, tag="ones_col")
            nc.gpsimd.memset(ones_col[:], 1.0)
            zbias = cpool.tile([P, 1], F32, tag="zbias")
            nc.gpsimd.memset(zbias[:], 0.0)
            id_f32 = cpool.tile([P, P], F32, tag="id_f32")
            make_identity(nc, id_f32)
            # token ids: tid_col[p, c] = p + 128*c
            tid_i = cpool.tile([P, NT], I32, tag="tid_i")
            nc.gpsimd.iota(tid_i[:], pattern=[[P, NT]], base=0,
                           channel_multiplier=1)
            tid_col = cpool.tile([P, NT], F32, tag="tid_col")
            nc.vector.tensor_copy(out=tid_col[:], in_=tid_i[:])
            zbig = cpool.tile([P, H], F32, tag="zbig")
            nc.gpsimd.memset(zbig[:], 0.0)
            wzero = cpool.tile([P, 512], BF16, tag="wzero")
            nc.gpsimd.memset(wzero[:], 0.0)
            ones_bf = cpool.tile([1, P], BF16, tag="ones_bf")
            nc.gpsimd.memset(ones_bf[:], 1.0)

            # --- constants for the on-chip count->offset prefix ---
            # flattening: column/partition index (t, e) -> t*8 + e
            # pmask[p=(j,e'), m=(i,e)] = (j < i) * (e' == e)
            tm_i = cpool.tile([P, P], I32, tag="tm_i")
            nc.gpsimd.iota(tm_i[:].rearrange("p (t e) -> p t e", e=E),
                           pattern=[[1, NT], [0, E]], base=0,
                           channel_multiplier=0)
            tm_row = cpool.tile([P, P], F32, tag="tm_row")
            nc.vector.tensor_copy(out=tm_row[:], in_=tm_i[:])
            em_i = cpool.tile([P, P], I32, tag="em_i")
            nc.gpsimd.iota(em_i[:].rearrange("p (t e) -> p t e", e=E),
                           pattern=[[0, NT], [1, E]], base=0,
                           channel_multiplier=0)
            em_row = cpool.tile([P, P], F32, tag="em_row")
            nc.vector.tensor_copy(out=em_row[:], in_=em_i[:])
            # tp_col/ep_col (per-partition t/e of index p) and pmask are
            # built inside phase 1: they need a PSUM pool for the K=1
            # transpose matmuls (engine memsets can't target partition
            # offsets that aren't 32-aligned)
            pmask = cpool.tile([P, P], F32, tag="pmask")
            # bases_row[0, (t,e)] = BASES[e] - 1
            bases_row = cpool.tile([1, P], F32, tag="bases_row")
            for e in range(E):
                nc.gpsimd.memset(
                    bases_row[:].rearrange("p (t e) -> p t e", e=E)
                    [:, :, e:e + 1], float(BASES[e] - 1))

            # dma_gather / dma_scatter_add ucode lives in the mlp library.
            nc.gpsimd.load_library(library_config.mlp)


            disp_sem = nc.alloc_semaphore("disp_dma")

            EORDER = sorted(range(E), key=lambda ee: -CAPS[ee])

            def prefetch_weights(e, ring):
                """Stage expert e's fc weights (+biases). Phase 1 uses the
                gpsimd ring (idle then); phase 2 the scalar ring (only
                activations there), keeping sync free for proj weights."""
                wfc_k = []
                for k in range(KH):
                    wk = wc.tile([P, F], BF16, tag=f"wfc{k}",
                                 name=f"wfc{k}_{e}")
                    ring.dma_start(
                        out=wk[:], in_=wfc.ap()[e, k * P:(k + 1) * P, :])
                    wfc_k.append(wk)
                bfc_sb = None
                bpj_sb = None
                if not zero_bias:
                    bfc_sb = wc.tile([P, KF], F32, tag="bfc_sb",
                                     name=f"bfc_{e}")
                    ring.dma_start(
                        out=bfc_sb[:],
                        in_=bfc.ap()[e:e + 1, :].rearrange(
                            "o (a p) -> p (o a)", p=P))
                    bpj_sb = wc.tile([1, H], BF16, tag="bpj_sb",
                                     name=f"bpj_{e}")
                    ring.dma_start(out=bpj_sb[:],
                                   in_=bpj.ap()[e:e + 1, :])
                return wfc_k, bfc_sb, bpj_sb

            # ============ PHASE 1: router + dispatch (batched) ============
            with tc.tile_pool(name="ph1", bufs=1) as ph1, \
                 tc.tile_pool(name="xtk", bufs=1) as xtkp, \
                 tc.tile_pool(name="ps_wu", bufs=2, space="PSUM") as ps_wu, \
                 tc.tile_pool(name="ps_lgt", bufs=1, space="PSUM") as ps_lgt, \
                 tc.tile_pool(name="ps_r", bufs=1, space="PSUM") as ps_r:
                # router inputs first on the sync ring (latency-critical)
                wr_sb = ph1.tile([P, KH * E], F32, tag="wr_sb")
                nc.sync.dma_start(
                    out=wr_sb[:, :].rearrange("p (k e) -> p k e", e=E),
                    in_=wr.ap()[:, :].rearrange("(k p) e -> p k e", p=P))
                # x chunks: one DMA per k so logits pipeline with the load
                xks = []
                for k in range(KH):
                    xk = xtkp.tile([P, TLOC], F32, tag=f"xk{k}")
                    ring = nc.sync if k % 2 == 0 else nc.scalar
                    ring.dma_start(
                        out=xk[:], in_=xt.ap()[k * P:(k + 1) * P, :])
                    xks.append(xk)
                if not zero_bias:
                    br_sb = ph1.tile([1, E], F32, tag="br_sb")
                    nc.sync.dma_start(out=br_sb[:], in_=brr.ap()[:, :])

                # ---- build pmask[p=(j,e'), m=(i,e)] = (j<i)*(e'==e):
                # transpose row 0 of the iota constants to per-partition
                # columns via K=1 matmuls, then two compares + a multiply
                pc_ps = ps_r.tile([P, 512], F32, tag="pr", name="pc_ps")
                nc.tensor.matmul(out=pc_ps[:, 0:1], lhsT=tm_row[0:1, :],
                                 rhs=ones_f[0:1, 0:1], start=True, stop=True)
                nc.tensor.matmul(out=pc_ps[:, 1:2], lhsT=em_row[0:1, :],
                                 rhs=ones_f[0:1, 0:1], start=True, stop=True)
                tp_col = ph1.tile([P, 2], F32, tag="tp_col")
                nc.vector.tensor_copy(out=tp_col[:], in_=pc_ps[:, 0:2])
                ptmp = ph1.tile([P, P], F32, tag="ptmp")
                nc.vector.tensor_scalar(out=pmask[:], in0=tm_row[:],
                                        scalar1=tp_col[:, 0:1], scalar2=None,
                                        op0=ALU.is_gt)
                nc.vector.tensor_scalar(out=ptmp[:], in0=em_row[:],
                                        scalar1=tp_col[:, 1:2], scalar2=None,
                                        op0=ALU.is_equal)
                nc.vector.tensor_mul(out=pmask[:], in0=pmask[:], in1=ptmp[:])

                def warm(n):
                    # keep the PE p-state ramped while inputs stream
                    for _ in range(n):
                        wps = ps_wu.tile([P, 512], F32, tag="wps")
                        nc.tensor.matmul(out=wps[:], lhsT=wzero[:, 0:P],
                                         rhs=wzero[:], start=True, stop=True)

                # init dispatch table + zero out (gpsimd ring: it is idle
                # during phase 1, and keeps sync/scalar free for the
                # dispatch critical path; WAW deps order these before the
                # scatter / scatter_add). Emitted BEFORE the weight
                # prefetches so the scatter isn't stuck behind them.
                for a0, aw in _chunks(NBLK, H // 64):
                    nc.gpsimd.dma_start(
                        out=wtbuf.ap()[:, :].rearrange(
                            "(a p) c -> p a c", p=P)[:, a0:a0 + aw, :],
                        in_=zbig[:, 0:aw * 64].rearrange(
                            "p (a c) -> p a c", c=64))
                for i in range(NT + 1):
                    nc.gpsimd.dma_start(out=out.ap()[i * P:(i + 1) * P, :],
                                        in_=zbig[:])

                # first two experts' fc weights don't depend on routing
                wstage = {EORDER[0]: prefetch_weights(EORDER[0], nc.gpsimd),
                          EORDER[1]: prefetch_weights(EORDER[1], nc.gpsimd)}

                # ---- logits, Wr-stationary: logitsT [8, 2048] accumulated
                # k-major in 4 segment banks, warmups interleaved per k-group
                # to absorb the per-chunk DMA cadence, then PE-transposed ----
                lgt_ps = [ps_lgt.tile([E, 512], F32, tag=f"lgt{s}",
                                      name=f"lgt{s}") for s in range(4)]
                warm(10)
                for k in range(KH):
                    for s in range(4):
                        nc.tensor.matmul(
                            out=lgt_ps[s][:, :],
                            lhsT=wr_sb[:, k * E:(k + 1) * E],
                            rhs=xks[k][:, s * 512:(s + 1) * 512],
                            start=(k == 0),
                            stop=(k == KH - 1 and zero_bias))
                    if k < KH - 1:
                        warm(8)
                if not zero_bias:
                    for s in range(4):
                        nc.tensor.matmul(
                            out=lgt_ps[s][:, :], lhsT=br_sb[0:1, :],
                            rhs=ones_f[0:1, 0:512],
                            start=False, stop=True)
                lgt_sb = ph1.tile([E, TLOC], F32, tag="lgt_sb")
                for s in range(4):
                    if s % 2 == 0:
                        nc.vector.tensor_copy(
                            out=lgt_sb[:, s * 512:(s + 1) * 512],
                            in_=lgt_ps[s][:, :])
                    else:
                        nc.scalar.copy(
                            out=lgt_sb[:, s * 512:(s + 1) * 512],
                            in_=lgt_ps[s][:, :])
                lg_ps = ps_r.tile([P, 512], F32, tag="pr", name="lg_ps")
                for i in range(NT):
                    nc.tensor.transpose(
                        out=lg_ps[:, i * E:(i + 1) * E],
                        in_=lgt_sb[0:E, i * P:(i + 1) * P],
                        identity=id_f32[0:E, 0:E])
                lg = ph1.tile([P, P], F32, tag="lg")
                nc.vector.tensor_copy(out=lg[:], in_=lg_ps[:, 0:P])

                def r3(t):  # [128, (16,8)] -> [128, 16, 8]
                    return t[:, :].rearrange("p (t e) -> p t e", e=E)

                # ---- top-2 (per tile), then batched compare/combine ----
                m8 = ph1.tile([P, P], F32, tag="m8")
                for i in range(NT):
                    nc.vector.max(out=m8[:, i * E:(i + 1) * E],
                                  in_=lg[:, i * E:(i + 1) * E])
                eq1 = ph1.tile([P, P], F32, tag="eq1")
                eq2 = ph1.tile([P, P], F32, tag="eq2")
                msk = ph1.tile([P, P], F32, tag="msk")
                nc.vector.tensor_tensor(
                    out=r3(eq1), in0=r3(lg),
                    in1=r3(m8)[:, :, 0:1].to_broadcast([P, NT, E]),
                    op=ALU.is_equal)
                nc.vector.tensor_tensor(
                    out=r3(eq2), in0=r3(lg),
                    in1=r3(m8)[:, :, 1:2].to_broadcast([P, NT, E]),
                    op=ALU.is_equal)
                nc.vector.tensor_add(out=msk[:], in0=eq1[:], in1=eq2[:])

                # combine weights: w1 = sigmoid(m1-m2), w2 = sigmoid(m2-m1)
                dt_ = ph1.tile([P, 2 * NT], F32, tag="dt_")
                d3 = dt_[:, :].rearrange("p (t s) -> p t s", s=2)
                nc.vector.tensor_sub(out=d3[:, :, 0:1],
                                     in0=r3(m8)[:, :, 0:1],
                                     in1=r3(m8)[:, :, 1:2])
                nc.vector.tensor_sub(out=d3[:, :, 1:2],
                                     in0=r3(m8)[:, :, 1:2],
                                     in1=r3(m8)[:, :, 0:1])
                wv = ph1.tile([P, 2 * NT], F32, tag="wv")
                nc.scalar.activation(out=wv[:], in_=dt_[:], func=AF.Sigmoid,
                                     bias=zbias[:])

                # ---- counts -> offsets, fully on-chip (no DRAM bounce) ----
                cnt_ps = ps_r.tile([1, 512], F32, tag="pr", name="cnt_ps")
                nc.tensor.matmul(out=cnt_ps[:, 0:P], lhsT=ones_col[:, 0:1],
                                 rhs=msk[:], start=True, stop=True)
                cnt_flat = ph1.tile([1, P], F32, tag="cnt_flat")
                nc.vector.tensor_copy(out=cnt_flat[:], in_=cnt_ps[:, 0:P])
                # transpose count row -> column via K=1 matmul
                cc_ps = ps_r.tile([P, 512], F32, tag="pr", name="cc_ps")
                nc.tensor.matmul(out=cc_ps[:, 0:1], lhsT=cnt_flat[0:1, :],
                                 rhs=ones_f[0:1, 0:1], start=True, stop=True)
                cnt_col = ph1.tile([P, 1], F32, tag="cnt_col")
                nc.vector.tensor_copy(out=cnt_col[:], in_=cc_ps[:, 0:1])
                # off_row[0, (i,e)] = sum_j cnt[(j,e)]*(j<i) + BASES[e]-1
                off_ps = ps_r.tile([1, 512], F32, tag="pr", name="off_ps")
                nc.tensor.matmul(out=off_ps[:, 0:P], lhsT=cnt_col[:, 0:1],
                                 rhs=pmask[:], start=True, stop=False)
                nc.tensor.matmul(out=off_ps[:, 0:P], lhsT=ones_f[0:1, 0:1],
                                 rhs=bases_row[:], start=False, stop=True)
                off_flat = ph1.tile([1, P], F32, tag="off_flat")
                nc.vector.tensor_copy(out=off_flat[:], in_=off_ps[:, 0:P])

                # ---- positions: within-tile inclusive prefix + offsets ----
                pos_ps = ps_r.tile([P, 512], F32, tag="pr", name="pos_ps")
                nc.tensor.matmul(out=pos_ps[:, 0:P], lhsT=u_incl[:], rhs=msk[:],
                                 start=True, stop=False)
                nc.tensor.matmul(out=pos_ps[:, 0:P], lhsT=ones_f[:, 0:P],
                                 rhs=off_flat[:], start=False, stop=True)

                tmp = ph1.tile([P, P], F32, tag="tmp")
                d1f = ph1.tile([P, NT], F32, tag="d1f")
                d2f = ph1.tile([P, NT], F32, tag="d2f")
                nc.vector.tensor_mul(out=tmp[:], in0=eq1[:], in1=pos_ps[:, 0:P])
                nc.vector.tensor_reduce(out=d1f[:], in_=r3(tmp), axis=AX.X,
                                        op=ALU.add)
                nc.vector.tensor_mul(out=tmp[:], in0=eq2[:], in1=pos_ps[:, 0:P])
                nc.vector.tensor_reduce(out=d2f[:], in_=r3(tmp), axis=AX.X,
                                        op=ALU.add)

                # ---- assemble scatter payload: dsi16 [128,(s,t)] indices,
                #      wp_all [128,(s,t),2] = (w, tid) records ----
                # allocated from the long-lived const pool: the scatter ucode
                # reads these via SDMA after the instruction retires, so their
                # SBUF space must not be recycled by the FFN pools
                dsi = cpool.tile([P, 2 * NT], I16, tag="dsi")
                nc.vector.tensor_copy(out=dsi[:, 0:NT], in_=d1f[:])
                nc.vector.tensor_copy(out=dsi[:, NT:2 * NT], in_=d2f[:])
                wp_all = cpool.tile([P, 2 * NT * 2], F32, tag="wp_all")
                w4 = wp_all[:, :].rearrange("p (s t c) -> p s t c", s=2, c=2)
                wv3 = wv[:, :].rearrange("p (t s) -> p t s", s=2)
                nc.vector.tensor_copy(
                    out=w4[:, 0:1, :, 0:1],
                    in_=wv3[:, :, 0:1].unsqueeze(1))
                nc.vector.tensor_copy(
                    out=w4[:, 1:2, :, 0:1],
                    in_=wv3[:, :, 1:2].unsqueeze(1))
                nc.vector.tensor_copy(
                    out=w4[:, 0:1, :, 1:2],
                    in_=tid_col[:, :].unsqueeze(1).unsqueeze(3))
                nc.vector.tensor_copy(
                    out=w4[:, 1:2, :, 1:2],
                    in_=tid_col[:, :].unsqueeze(1).unsqueeze(3))

                # ---- wrapped-16 index staging for the scatter:
                # record q = r*128 + p  ->  idxs[q%16, q//16], i.e.
                # PD2[p%16, 8*r + p//16] = dsi[p, r]. 8 strided writes
                # spread over sync/scalar/gpsimd rings (all free here),
                # then contiguous reads replicated into each group.
                pd2v = pd2.ap()[:, :].rearrange("pw (r g) -> pw r g", g=E)
                for g in range(P // 16):
                    ring = nc.sync if g % 2 == 0 else nc.scalar
                    ring.dma_start(
                        out=pd2v[:, :, g:g + 1],
                        in_=dsi[16 * g:16 * (g + 1), :].unsqueeze(2))
                idx_disp = cpool.tile([P, NCOL], I16, tag="idx_disp")
                for g in range(P // 16):
                    nc.scalar.dma_start(out=idx_disp[16 * g:16 * (g + 1), :],
                                        in_=pd2.ap()[:, :])
                nc.gpsimd.dma_scatter_add(
                    out_ap=wtbuf.ap()[:, 0:2],
                    in_ap=wp_all[:, :].rearrange("p (r c) -> p r c", c=2),
                    idxs_ap=idx_disp[:, :], num_idxs=2 * TLOC,
                    num_idxs_reg=2 * TLOC, elem_size=2,
                    elem_step=64).then_inc(disp_sem, 16)
                nc.gpsimd.wait_ge(disp_sem, 16)

            # ================= PHASE 2: expert FFN + combine ==============
            with tc.tile_pool(name="wpp", bufs=6) as wpp, \
                 tc.tile_pool(name="xet", bufs=2) as xetp, \
                 tc.tile_pool(name="h1t", bufs=1) as h1tp, \
                 tc.tile_pool(name="ysb", bufs=2) as ysbp, \
                 tc.tile_pool(name="wtk", bufs=2) as wtkp, \
                 tc.tile_pool(name="ps_s1", bufs=2, space="PSUM") as ps_s1, \
                 tc.tile_pool(name="ps_st", bufs=2, space="PSUM") as ps_st, \
                 tc.tile_pool(name="ps_y", bufs=1, space="PSUM") as ps_y:
                # the scatter ucode's DRAM write isn't tracked as a wtbuf
                # write by Tile: explicitly gate the wtbuf readers
                nc.sync.wait_ge(disp_sem, 16)
                nc.scalar.wait_ge(disp_sem, 16)

                def prefetch_disp(e, md_ring=nc.sync):
                    """Stage expert e's dispatch metadata and routed tokens.
                    The wrapped tid read is element-granular (~4us transfer),
                    so it runs ONCE into 16 partitions on the gpsimd ring;
                    the tiny i16 result is replicated to all 8 gpsimd-core
                    groups via a contiguous DRAM bounce. All reads here live
                    on gpsimd/scalar so a pending wait never blocks the
                    sync-ring proj weight stream (DMA queues are FIFO)."""
                    CAP, CAPR, BASE = CAPS[e], CAPRS[e], BASES[e]
                    NW = CAPR // 16
                    SUBS = _chunks(CAP, P)
                    tidw_f = wtkp.tile([P, NW], F32, tag="tidw_f",
                                       name=f"tidw_{e}")
                    nc.gpsimd.dma_start(
                        out=tidw_f[0:16, :],
                        in_=wtbuf.ap()[BASE:BASE + CAPR, 1:2].rearrange(
                            "(s p) o -> p (s o)", p=16))
                    idx16 = wtkp.tile([P, NW], I16, tag="idx16",
                                      name=f"idx16_{e}")
                    nc.vector.tensor_copy(out=idx16[0:16, :],
                                          in_=tidw_f[0:16, :])
                    nc.gpsimd.dma_start(out=pdx.ap()[e, :, 0:NW],
                                        in_=idx16[0:16, :])
                    idx = wtkp.tile([P, NW], I16, tag="idx",
                                    name=f"idx_{e}")
                    for g in range(P // 16):
                        md_ring.dma_start(out=idx[16 * g:16 * (g + 1), :],
                                          in_=pdx.ap()[e, :, 0:NW])
                    w_t = []
                    for ci, (cs, cw) in enumerate(SUBS):
                        wt = wtkp.tile([P, 1], F32, tag=f"wt{ci}",
                                       name=f"wt{ci}_{e}")
                        md_ring.dma_start(
                            out=wt[0:cw, :],
                            in_=wtbuf.ap()[BASE + cs:BASE + cs + cw, 0:1])
                        w_t.append(wt)
                    # gather x rows transposed:
                    # xet3d[p, k, c] = x[tid[c], k*128+p]
                    xet3d = xetp.tile([P, KH * CAPR], BF16, tag="xet3d",
                                      name=f"xet_{e}")
                    nc.gpsimd.dma_gather(
                        out_ap=xet3d[:, :].rearrange("p (k c) -> p k c", k=KH),
                        in_ap=xbf.ap()[:, :],
                        idxs_ap=idx[:, :], num_idxs=CAPR, num_idxs_reg=CAPR,
                        elem_size=H, transpose=True)
                    return idx, w_t, xet3d

                dstage = {EORDER[0]: prefetch_disp(EORDER[0],
                                                    md_ring=nc.scalar)}
                # re-ramp the PE p-state during the dispatch tail: wramp is
                # memset on the vector engine AFTER expert 0's idx cast, so
                # these warmups execute right before expert 0's fc
                wramp = cpool.tile([P, 512], BF16, tag="wramp")
                nc.vector.memset(wramp[:], 0.0)
                for wi in range(24):
                    wps2 = ps_s1.tile([P, 512], F32, tag="ps1",
                                      name=f"wps2_{wi}")
                    nc.tensor.matmul(out=wps2[:], lhsT=wramp[:, 0:P],
                                     rhs=wramp[:], start=True, stop=True)
                for ei, e in enumerate(EORDER):
                    CAP, CAPR, BASE = CAPS[e], CAPRS[e], BASES[e]
                    assert CAP <= 640
                    SUBS = _chunks(CAP, P)
                    NSUB = len(SUBS)
                    NSUBM = min(NSUB, 4)
                    TAILW = CAP - 512 if CAP > 512 else 0
                    wfc_k, bfc_sb, bpj_sb = wstage.pop(e)
                    idx, w_t, xet3d = dstage.pop(e)
                    # prefetch next expert's dispatch (gather overlaps fc)
                    if ei + 1 < E:
                        dstage[EORDER[ei + 1]] = prefetch_disp(EORDER[ei + 1])

                    # ---------------- fc: h1 = gelu(x @ Wfc) ----------------
                    # CAP <= 512: single 512-wide segment (one PSUM bank).
                    # CAP > 512: split at 256 so both segments are >= 107ns
                    # of PE rows (no LDWEIGHTS stall on tiny tail matmuls).
                    # Segment list: (col0, width, h1 tile, tile stride)
                    h1all = h1tp.tile([P, KF * 512], BF16, tag="h1all",
                                      name=f"h1_{e}")
                    h1tl = None
                    if TAILW:
                        h1tl = h1tp.tile([P, KF * 384], BF16, tag="h1tl",
                                         name=f"h1t_{e}")
                        FSEG = [(0, 256, h1all, 512, ps_s1),
                                (256, CAP - 256, h1tl, 384, ps_st)]
                    else:
                        FSEG = [(0, CAP, h1all, 512, ps_s1)]
                    for ft in range(KF):
                        pss = []
                        for si, (_, _, _, _, pool) in enumerate(FSEG):
                            ptag = "ps1" if pool is ps_s1 else "pst"
                            pss.append(pool.tile([P, 512], F32, tag=ptag,
                                                 name=f"pss{si}"))
                        for k in range(KH):
                            for si, (c0, cwid, _, _, _) in enumerate(FSEG):
                                nc.tensor.matmul(
                                    out=pss[si][:, 0:cwid],
                                    lhsT=wfc_k[k][:, ft * P:(ft + 1) * P],
                                    rhs=xet3d[:, k * CAPR + c0:
                                              k * CAPR + c0 + cwid],
                                    start=(k == 0), stop=(k == KH - 1))
                        bias_ap = (zbias[:] if zero_bias
                                   else bfc_sb[:, ft:ft + 1])
                        for si, (c0, cwid, h1t_, hstride, _) in \
                                enumerate(FSEG):
                            nc.scalar.activation(
                                out=h1t_[:, ft * hstride:ft * hstride + cwid],
                                in_=pss[si][:, 0:cwid],
                                func=_ACT_FN, bias=bias_ap)

                    def h1_slice(k, cs, cw):
                        """lhsT slice for proj sub [cs, cs+cw)."""
                        if TAILW and cs >= 256:
                            return h1tl[:, k * 384 + cs - 256:
                                        k * 384 + cs - 256 + cw]
                        return h1all[:, k * 512 + cs:k * 512 + cs + cw]

                    # fc weights for e are no longer needed: stream e+2's now
                    # (scalar ring; WAR on the last fc matmul read above)
                    if ei + 2 < E:
                        wstage[EORDER[ei + 2]] = prefetch_weights(
                            EORDER[ei + 2], nc.scalar)

                    # ------------- proj: y = w * (h1 @ Wproj) ---------------
                    ysb = ysbp.tile([P, NSUB * H], F32, tag="ysb",
                                    name=f"ysb_{e}")
                    if CAP % P:
                        # zero the partial last sub's block: its rows cw:128
                        # are scatter-added (to the pad token) and must be 0
                        nc.vector.memset(
                            ysb[0:P, (NSUB - 1) * H:NSUB * H], 0.0)
                    for hi, (hs, hw) in enumerate(HT):
                        psy = [ps_y.tile([P, 512], F32, tag=f"psy{ci}",
                                         name=f"psy{ci}{hi}_{e}")
                               for ci in range(NSUBM)]
                        ptl = None
                        if TAILW:
                            ptl = ps_st.tile([P, 512], F32, tag="pst",
                                             name=f"ptl{hi}_{e}")
                        for k2 in range(KF // 2):
                            # load TWO k-chunks per DMA: halves the issue
                            # count so the ring keeps pace with the PE in
                            # the 256-wide pass
                            wp = wpp.tile([P, 2 * 512], BF16, tag="wp")
                            nc.sync.dma_start(
                                out=wp[:, :].rearrange(
                                    "p (b c) -> p b c", b=2)[:, :, 0:hw],
                                in_=wpj.ap()[e, 2 * k2 * P:
                                             (2 * k2 + 2) * P,
                                             hs:hs + hw].rearrange(
                                    "(b p) c -> p b c", p=P))
                            for b in range(2):
                                k = 2 * k2 + b
                                wps = wp[:, b * 512:b * 512 + hw]
                                for ci in range(NSUBM):
                                    cs, cw = SUBS[ci]
                                    nc.tensor.matmul(
                                        out=psy[ci][0:cw, 0:hw],
                                        lhsT=h1_slice(k, cs, cw),
                                        rhs=wps,
                                        start=(k == 0),
                                        stop=(zero_bias and k == KF - 1))
                                if TAILW:
                                    cs, cw = SUBS[4]
                                    nc.tensor.matmul(
                                        out=ptl[0:cw, 0:hw],
                                        lhsT=h1_slice(k, cs, cw),
                                        rhs=wps,
                                        start=(k == 0),
                                        stop=(zero_bias and k == KF - 1))
                        for ci in range(NSUB):
                            cs, cw = SUBS[ci]
                            pt = ptl if ci == 4 else psy[ci]
                            if not zero_bias:
                                nc.tensor.matmul(
                                    out=pt[0:cw, 0:hw],
                                    lhsT=ones_bf[:, 0:cw],
                                    rhs=bpj_sb[:, hs:hs + hw],
                                    start=False, stop=True)
                            nc.vector.tensor_scalar_mul(
                                ysb[0:cw, ci * H + hs:ci * H + hs + hw],
                                pt[0:cw, 0:hw],
                                w_t[ci][0:cw, 0:1])
                    # one scatter-add per expert: per-sub ucode calls
                    # serialize at ~5.7us each on the gpsimd queue
                    nc.gpsimd.dma_scatter_add(
                        out_ap=out.ap()[:, :],
                        in_ap=ysb[:, :].rearrange("p (n h) -> p n h", n=NSUB),
                        idxs_ap=idx[:, :], num_idxs=CAPR, num_idxs_reg=CAPR,
                        elem_size=H)

    nc.compile()
    return nc


# ---------------------------------------------------------------------------
_BUILD_CACHE = {}
_LAST_IN_MAPS = None


def _get_built(TLOC, H, F, E, CAPS, zero_bias=False):
    key = (TLOC, H, F, E, tuple(CAPS), zero_bias)
    if key not in _BUILD_CACHE:
        _BUILD_CACHE[key] = build_moe(TLOC, H, F, E, tuple(CAPS),
                                      zero_bias=zero_bias)
    return _BUILD_CACHE[key]


def kernel(hidden_states, Wr, br, Wfc, bfc, Wproj, bproj):
    from concourse.bass_utils import run_bass_kernel_spmd

    hs = np.ascontiguousarray(np.asarray(hidden_states, dtype=np.float32))
    Wr = np.ascontiguousarray(np.asarray(Wr, dtype=np.float32))
    br = np.ascontiguousarray(np.asarray(br, dtype=np.float32))
    Wfc = np.asarray(Wfc, dtype=np.float32)
    bfc = np.ascontiguousarray(np.asarray(bfc, dtype=np.float32))
    Wproj = np.asarray(Wproj, dtype=np.float32)
    bproj = np.asarray(bproj, dtype=np.float32)

    B, S, H = hs.shape
    E, H2, F = Wfc.shape
    assert H2 == H
    T = B * S
    assert T % N_CORES == 0
    TLOC = T // N_CORES
    x = hs.reshape(T, H)

    # host-side routing peek ONLY to pick compile-time capacities and the
    # balanced token->core assignment (routing itself runs on device)
    logits = x @ Wr + br[None, :]
    top2 = np.argpartition(-logits, 2, axis=1)[:, :2]
    pair_lo = np.minimum(top2[:, 0], top2[:, 1])
    pair_hi = np.maximum(top2[:, 0], top2[:, 1])
    type_id = pair_lo.astype(np.int64) * E + pair_hi
    order = np.argsort(type_id, kind="stable")
    # deal the type-sorted token list round-robin: core g takes order[g::8]
    core_toks = [order[g::N_CORES] for g in range(N_CORES)]
    maxcnt = np.zeros(E, dtype=np.int64)
    for g in range(N_CORES):
        cnts = np.bincount(top2[core_toks[g]].ravel(), minlength=E)
        maxcnt = np.maximum(maxcnt, cnts)
    CAPS = tuple(int(max(128, math.ceil((m + 4) / 8.0) * 8))
                 for m in maxcnt)

    zero_bias = bool(np.all(bfc == 0.0) and np.all(bproj == 0.0)
                     and np.all(br == 0.0))
    nc = _get_built(TLOC, H, F, E, CAPS, zero_bias)

    wfc_bf = np.ascontiguousarray(Wfc.astype(ml_dtypes.bfloat16))
    wpj_bf = np.ascontiguousarray(Wproj.astype(ml_dtypes.bfloat16))

    in_maps = []
    for g in range(N_CORES):
        xc = x[core_toks[g]]
        xbf_c = np.zeros((TLOC + 128, H), dtype=ml_dtypes.bfloat16)
        xbf_c[:TLOC] = xc.astype(ml_dtypes.bfloat16)
        m = {
            "xt": np.ascontiguousarray(xc.T),
            "xbf": xbf_c,
            "wr": Wr,
            "wfc": wfc_bf,
            "wpj": wpj_bf,
        }
        if not zero_bias:
            m["brr"] = np.ascontiguousarray(br.reshape(1, E))
            m["bfc"] = bfc
            m["bpj"] = np.ascontiguousarray(bproj.astype(ml_dtypes.bfloat16))
        in_maps.append(m)

    global _LAST_IN_MAPS
    _LAST_IN_MAPS = in_maps

    res = run_bass_kernel_spmd(nc, in_maps, core_ids=list(range(N_CORES)))
    full = np.empty((T, H), dtype=np.float32)
    for g in range(N_CORES):
        full[core_toks[g]] = res.results[g]["out"][:TLOC]
    return full.reshape(B, S, H).astype(np.float32)


# revision 48
# speedup vs baseline: 1.2197x; 1.2197x over previous
"""MoE (8 experts, top-2) Bass kernel for 8 trn2 NeuronCores.

Strategy: data-parallel over tokens with HOST-BALANCED token->core assignment.
The host groups tokens by their top-2 expert pair (type) and deals them
round-robin across the 8 cores, so every core sees ~C_e/8 tokens per expert.
That pins the per-expert capacity CAP_e at ~512, which removes the padded
5th proj sub-tile and the fc tail matmuls that dominate PE waste when
capacities are unbalanced.

On device, per core:
  phase 1: router logits via fp32 matmuls on host-transposed x chunks
           (per-k-chunk DMAs pipelined with PE warmup), batched top-2 /
           combine-weight math, counts -> offsets via an on-chip prefix
           matmul (constant prefix mask; no DRAM round-trips), positions,
           then ONE (w, token_id) record scatter into the position-ordered
           dispatch table. The scatter's wrapped-16 index layout is staged
           through DRAM with 8 small strided writes spread over all three
           DMA rings + contiguous replicated readbacks, replacing the
           baseline's 8 full strided re-reads (~70us -> ~15us).
  phase 2: per expert: dma_gather(transpose=True) routed token rows into
           [H-chunk, c] matmul layout, dense FFN gelu(x@Wfc)@Wproj in bf16
           with fp32 accumulate (fc split at 256 for over-512 capacities so
           no matmul is LDWEIGHTS-bound), scale rows by combine weight, one
           dma_scatter_add per expert. Ring discipline (DMA queues are FIFO
           and a waiting dma_start blocks its ENGINE): sync carries only the
           paired proj-weight stream + expert metadata whose waits have
           slack; scalar carries activations + fc weight prefetch (2 experts
           ahead; during phase 1 on the idle gpsimd ring); gpsimd carries
           zero-init, the wrapped tid read (element-granular), gathers and
           scatters. The wrapped gather indices are read once into 16
           partitions, cast to i16, and replicated to all 8 gpsimd-core
           groups via a tiny contiguous DRAM bounce. PE p-state is re-ramped
           with dependency-gated warmup matmuls right before expert 0.

Host does only slicing/concat/layout staging (permutation + transpose +
bf16 cast) plus a routing peek to pick compile-time capacities and the
balanced assignment.
"""

import math
import os
import sys

import numpy as np

for _p in ("/opt/trn_rl_repo", "/root/.axon_site/_ro/trn_rl_repo"):
    if os.path.isdir(_p) and _p not in sys.path:
        sys.path.insert(0, _p)

import ml_dtypes  # noqa: E402
import concourse.bass as bass  # noqa: E402
import concourse.mybir as mybir  # noqa: E402
import concourse.tile as tile  # noqa: E402
from concourse import bacc  # noqa: E402
from concourse.masks import make_upper_triangular, make_identity  # noqa: E402
from concourse import library_config  # noqa: E402

F32 = mybir.dt.float32
BF16 = mybir.dt.bfloat16
I32 = mybir.dt.int32
I16 = mybir.dt.int16
AF = mybir.ActivationFunctionType
ALU = mybir.AluOpType
AX = mybir.AxisListType

N_CORES = 8
P = 128
_ACT_FN = AF.Gelu_apprx_tanh  # debug hook: sim lacks gelu, tests swap in Tanh


def _chunks(total, step):
    out = []
    off = 0
    while off < total:
        w = min(step, total - off)
        out.append((off, w))
        off += w
    return out


def build_moe(TLOC, H, F, E, CAPS, zero_bias=False):
    """Build the per-core Bass program (SPMD: identical on all cores)."""
    assert TLOC % P == 0 and H % P == 0 and F % P == 0 and E == 8
    CAPS = list(CAPS)
    assert len(CAPS) == E and all(c % 8 == 0 for c in CAPS)
    CAPRS = [((c + P - 1) // P) * P for c in CAPS]
    KH = H // P            # contraction chunks over H (6)
    KF = F // P            # f-tiles (and stage-2 contraction chunks) (24)
    NT = TLOC // P         # token tiles (16)
    assert NT == 16
    BASES = [sum(CAPRS[:e]) for e in range(E)]
    NPOS = sum(CAPRS) + P
    NPOS = ((NPOS + P - 1) // P) * P
    NBLK = NPOS // P
    HT = ((0, 512), (512, H - 512))  # proj h-tiles

    nc = bacc.Bacc("TRN2", target_bir_lowering=False, debug=False,
                   enable_asserts=True, num_devices=N_CORES)

    xt = nc.dram_tensor("xt", [H, TLOC], F32, kind="ExternalInput")
    xbf = nc.dram_tensor("xbf", [TLOC + P, H], BF16, kind="ExternalInput")
    wr = nc.dram_tensor("wr", [H, E], F32, kind="ExternalInput")
    wfc = nc.dram_tensor("wfc", [E, H, F], BF16, kind="ExternalInput")
    wpj = nc.dram_tensor("wpj", [E, F, H], BF16, kind="ExternalInput")
    if not zero_bias:
        brr = nc.dram_tensor("brr", [1, E], F32, kind="ExternalInput")
        bfc = nc.dram_tensor("bfc", [E, F], F32, kind="ExternalInput")
        bpj = nc.dram_tensor("bpj", [E, H], BF16, kind="ExternalInput")
    # extra trash tile rows at the end absorb pad-slot scatter-adds
    out = nc.dram_tensor("out", [TLOC + P, H], F32, kind="ExternalOutput")

    # dispatch table: (combine w, token id) in cols 0:2 of 256B-strided rows
    # (dma_scatter_add needs a 256B row stride)
    wtbuf = nc.dram_tensor("wtbuf", [NPOS, 64], F32)
    # wrapped-16 dispatch index staging: PD2[p%16, 8*r + p//16] = dsi[p, r]
    NCOL = 2 * TLOC // 16
    pd2 = nc.dram_tensor("pd2", [16, NCOL], I16)
    # per-expert gather-index replication bounce (i16, wrap-16 layout)
    MAXW = max(CAPRS) // 16
    pdx = nc.dram_tensor("pdx", [E, 16, MAXW], I16)

    with tile.TileContext(nc) as tc:
        with tc.tile_pool(name="const", bufs=1) as cpool, \
             tc.tile_pool(name="wc", bufs=2) as wc:
            # ---------------- constants ----------------
            u_incl = cpool.tile([P, P], F32, tag="u_incl")
            make_upper_triangular(nc, u_incl, val=1.0, diag=True)
            ones_f = cpool.tile([1, 512], F32, tag="ones_f")
            nc.gpsimd.memset(ones_f[:], 1.0)
            ones_col = cpool.tile([P, 1], F32, tag="ones_col")
            nc.gpsimd.memset(ones_col[:], 1.0)
            zbias = cpool.tile([P, 1], F32, tag="zbias")
            nc.gpsimd.memset(zbias[:], 0.0)
            id_f32 = cpool.tile([P, P], F32, tag="id_f32")
            make_identity(nc, id_f32)
            # token ids: tid_col[p, c] = p + 128*c
            tid_i = cpool.tile([P, NT], I32, tag="tid_i")
            nc.gpsimd.iota(tid_i[:], pattern=[[P, NT]], base=0,
                           channel_multiplier=1)
            tid_col = cpool.tile([P, NT], F32, tag="tid_col")
            nc.vector.tensor_copy(out=tid_col[:], in_=tid_i[:])
            zbig = cpool.tile([P, H], F32, tag="zbig")
            nc.gpsimd.memset(zbig[:], 0.0)
            wzero = cpool.tile([P, 512], BF16, tag="wzero")
            nc.gpsimd.memset(wzero[:], 0.0)
            ones_bf = cpool.tile([1, P], BF16, tag="ones_bf")
            nc.gpsimd.memset(ones_bf[:], 1.0)

            # --- constants for the on-chip count->offset prefix ---
            # flattening: column/partition index (t, e) -> t*8 + e
            # pmask[p=(j,e'), m=(i,e)] = (j < i) * (e' == e)
            tm_i = cpool.tile([P, P], I32, tag="tm_i")
            nc.gpsimd.iota(tm_i[:].rearrange("p (t e) -> p t e", e=E),
                           pattern=[[1, NT], [0, E]], base=0,
                           channel_multiplier=0)
            tm_row = cpool.tile([P, P], F32, tag="tm_row")
            nc.vector.tensor_copy(out=tm_row[:], in_=tm_i[:])
            em_i = cpool.tile([P, P], I32, tag="em_i")
            nc.gpsimd.iota(em_i[:].rearrange("p (t e) -> p t e", e=E),
                           pattern=[[0, NT], [1, E]], base=0,
                           channel_multiplier=0)
            em_row = cpool.tile([P, P], F32, tag="em_row")
            nc.vector.tensor_copy(out=em_row[:], in_=em_i[:])
            # tp_col/ep_col (per-partition t/e of index p) and pmask are
            # built inside phase 1: they need a PSUM pool for the K=1
            # transpose matmuls (engine memsets can't target partition
            # offsets that aren't 32-aligned)
            pmask = cpool.tile([P, P], F32, tag="pmask")
            # bases_row[0, (t,e)] = BASES[e] - 1
            bases_row = cpool.tile([1, P], F32, tag="bases_row")
            for e in range(E):
                nc.gpsimd.memset(
                    bases_row[:].rearrange("p (t e) -> p t e", e=E)
                    [:, :, e:e + 1], float(BASES[e] - 1))

            # dma_gather / dma_scatter_add ucode lives in the mlp library.
            nc.gpsimd.load_library(library_config.mlp)


            disp_sem = nc.alloc_semaphore("disp_dma")

            EORDER = sorted(range(E), key=lambda ee: -CAPS[ee])

            def prefetch_weights(e, ring):
                """Stage expert e's fc weights (+biases). Phase 1 uses the
                gpsimd ring (idle then); phase 2 the scalar ring (only
                activations there), keeping sync free for proj weights."""
                wfc_k = []
                for k in range(KH):
                    wk = wc.tile([P, F], BF16, tag=f"wfc{k}",
                                 name=f"wfc{k}_{e}")
                    ring.dma_start(
                        out=wk[:], in_=wfc.ap()[e, k * P:(k + 1) * P, :])
                    wfc_k.append(wk)
                bfc_sb = None
                bpj_sb = None
                if not zero_bias:
                    bfc_sb = wc.tile([P, KF], F32, tag="bfc_sb",
                                     name=f"bfc_{e}")
                    ring.dma_start(
                        out=bfc_sb[:],
                        in_=bfc.ap()[e:e + 1, :].rearrange(
                            "o (a p) -> p (o a)", p=P))
                    bpj_sb = wc.tile([1, H], BF16, tag="bpj_sb",
                                     name=f"bpj_{e}")
                    ring.dma_start(out=bpj_sb[:],
                                   in_=bpj.ap()[e:e + 1, :])
                return wfc_k, bfc_sb, bpj_sb

            # ============ PHASE 1: router + dispatch (batched) ============
            with tc.tile_pool(name="ph1", bufs=1) as ph1, \
                 tc.tile_pool(name="xtk", bufs=1) as xtkp, \
                 tc.tile_pool(name="ps_wu", bufs=2, space="PSUM") as ps_wu, \
                 tc.tile_pool(name="ps_lgt", bufs=1, space="PSUM") as ps_lgt, \
                 tc.tile_pool(name="ps_r", bufs=1, space="PSUM") as ps_r:
                # router inputs first on the sync ring (latency-critical)
                wr_sb = ph1.tile([P, KH * E], F32, tag="wr_sb")
                nc.sync.dma_start(
                    out=wr_sb[:, :].rearrange("p (k e) -> p k e", e=E),
                    in_=wr.ap()[:, :].rearrange("(k p) e -> p k e", p=P))
                # x chunks: one DMA per k so logits pipeline with the load
                xks = []
                for k in range(KH):
                    xk = xtkp.tile([P, TLOC], F32, tag=f"xk{k}")
                    ring = nc.sync if k % 2 == 0 else nc.scalar
                    ring.dma_start(
                        out=xk[:], in_=xt.ap()[k * P:(k + 1) * P, :])
                    xks.append(xk)
                if not zero_bias:
                    br_sb = ph1.tile([1, E], F32, tag="br_sb")
                    nc.sync.dma_start(out=br_sb[:], in_=brr.ap()[:, :])

                # ---- build pmask[p=(j,e'), m=(i,e)] = (j<i)*(e'==e):
                # transpose row 0 of the iota constants to per-partition
                # columns via K=1 matmuls, then two compares + a multiply
                pc_ps = ps_r.tile([P, 512], F32, tag="pr", name="pc_ps")
                nc.tensor.matmul(out=pc_ps[:, 0:1], lhsT=tm_row[0:1, :],
                                 rhs=ones_f[0:1, 0:1], start=True, stop=True)
                nc.tensor.matmul(out=pc_ps[:, 1:2], lhsT=em_row[0:1, :],
                                 rhs=ones_f[0:1, 0:1], start=True, stop=True)
                tp_col = ph1.tile([P, 2], F32, tag="tp_col")
                nc.vector.tensor_copy(out=tp_col[:], in_=pc_ps[:, 0:2])
                ptmp = ph1.tile([P, P], F32, tag="ptmp")
                nc.vector.tensor_scalar(out=pmask[:], in0=tm_row[:],
                                        scalar1=tp_col[:, 0:1], scalar2=None,
                                        op0=ALU.is_gt)
                nc.vector.tensor_scalar(out=ptmp[:], in0=em_row[:],
                                        scalar1=tp_col[:, 1:2], scalar2=None,
                                        op0=ALU.is_equal)
                nc.vector.tensor_mul(out=pmask[:], in0=pmask[:], in1=ptmp[:])

                def warm(n):
                    # keep the PE p-state ramped while inputs stream
                    for _ in range(n):
                        wps = ps_wu.tile([P, 512], F32, tag="wps")
                        nc.tensor.matmul(out=wps[:], lhsT=wzero[:, 0:P],
                                         rhs=wzero[:], start=True, stop=True)

                # init dispatch table + zero out (gpsimd ring: it is idle
                # during phase 1, and keeps sync/scalar free for the
                # dispatch critical path; WAW deps order these before the
                # scatter / scatter_add). Emitted BEFORE the weight
                # prefetches so the scatter isn't stuck behind them.
                for a0, aw in _chunks(NBLK, H // 64):
                    nc.gpsimd.dma_start(
                        out=wtbuf.ap()[:, :].rearrange(
                            "(a p) c -> p a c", p=P)[:, a0:a0 + aw, :],
                        in_=zbig[:, 0:aw * 64].rearrange(
                            "p (a c) -> p a c", c=64))
                for i in range(NT + 1):
                    nc.gpsimd.dma_start(out=out.ap()[i * P:(i + 1) * P, :],
                                        in_=zbig[:])

                # first two experts' fc weights don't depend on routing
                wstage = {EORDER[0]: prefetch_weights(EORDER[0], nc.gpsimd),
                          EORDER[1]: prefetch_weights(EORDER[1], nc.gpsimd)}

                # ---- logits, Wr-stationary: logitsT [8, 2048] accumulated
                # k-major in 4 segment banks, warmups interleaved per k-group
                # to absorb the per-chunk DMA cadence, then PE-transposed ----
                # [16, 512] so the banks can be reused by the wrap-staging
                # block transposes after the logits drain (logits use rows
                # 0:8 only)
                lgt_ps = [ps_lgt.tile([16, 512], F32, tag=f"lgt{s}",
                                      name=f"lgt{s}") for s in range(4)]
                warm(10)
                for k in range(KH):
                    for s in range(4):
                        nc.tensor.matmul(
                            out=lgt_ps[s][0:E, :],
                            lhsT=wr_sb[:, k * E:(k + 1) * E],
                            rhs=xks[k][:, s * 512:(s + 1) * 512],
                            start=(k == 0),
                            stop=(k == KH - 1 and zero_bias))
                    if k < KH - 1:
                        warm(8)
                if not zero_bias:
                    for s in range(4):
                        nc.tensor.matmul(
                            out=lgt_ps[s][0:E, :], lhsT=br_sb[0:1, :],
                            rhs=ones_f[0:1, 0:512],
                            start=False, stop=True)
                lgt_sb = ph1.tile([E, TLOC], F32, tag="lgt_sb")
                for s in range(4):
                    if s % 2 == 0:
                        nc.vector.tensor_copy(
                            out=lgt_sb[:, s * 512:(s + 1) * 512],
                            in_=lgt_ps[s][0:E, :])
                    else:
                        nc.scalar.copy(
                            out=lgt_sb[:, s * 512:(s + 1) * 512],
                            in_=lgt_ps[s][0:E, :])
                lg_ps = ps_r.tile([P, 512], F32, tag="pr", name="lg_ps")
                for i in range(NT):
                    nc.tensor.transpose(
                        out=lg_ps[:, i * E:(i + 1) * E],
                        in_=lgt_sb[0:E, i * P:(i + 1) * P],
                        identity=id_f32[0:E, 0:E])
                lg = ph1.tile([P, P], F32, tag="lg")
                nc.vector.tensor_copy(out=lg[:], in_=lg_ps[:, 0:P])

                def r3(t):  # [128, (16,8)] -> [128, 16, 8]
                    return t[:, :].rearrange("p (t e) -> p t e", e=E)

                # ---- top-2 (per tile), then batched compare/combine ----
                m8 = ph1.tile([P, P], F32, tag="m8")
                for i in range(NT):
                    nc.vector.max(out=m8[:, i * E:(i + 1) * E],
                                  in_=lg[:, i * E:(i + 1) * E])
                eq1 = ph1.tile([P, P], F32, tag="eq1")
                eq2 = ph1.tile([P, P], F32, tag="eq2")
                msk = ph1.tile([P, P], F32, tag="msk")
                nc.vector.tensor_tensor(
                    out=r3(eq1), in0=r3(lg),
                    in1=r3(m8)[:, :, 0:1].to_broadcast([P, NT, E]),
                    op=ALU.is_equal)
                nc.vector.tensor_tensor(
                    out=r3(eq2), in0=r3(lg),
                    in1=r3(m8)[:, :, 1:2].to_broadcast([P, NT, E]),
                    op=ALU.is_equal)
                nc.vector.tensor_add(out=msk[:], in0=eq1[:], in1=eq2[:])

                # combine weights: w1 = sigmoid(m1-m2), w2 = sigmoid(m2-m1)
                dt_ = ph1.tile([P, 2 * NT], F32, tag="dt_")
                d3 = dt_[:, :].rearrange("p (t s) -> p t s", s=2)
                nc.vector.tensor_sub(out=d3[:, :, 0:1],
                                     in0=r3(m8)[:, :, 0:1],
                                     in1=r3(m8)[:, :, 1:2])
                nc.vector.tensor_sub(out=d3[:, :, 1:2],
                                     in0=r3(m8)[:, :, 1:2],
                                     in1=r3(m8)[:, :, 0:1])
                wv = ph1.tile([P, 2 * NT], F32, tag="wv")
                nc.scalar.activation(out=wv[:], in_=dt_[:], func=AF.Sigmoid,
                                     bias=zbias[:])

                # ---- counts -> offsets, fully on-chip (no DRAM bounce) ----
                cnt_ps = ps_r.tile([1, 512], F32, tag="pr", name="cnt_ps")
                nc.tensor.matmul(out=cnt_ps[:, 0:P], lhsT=ones_col[:, 0:1],
                                 rhs=msk[:], start=True, stop=True)
                cnt_flat = ph1.tile([1, P], F32, tag="cnt_flat")
                nc.vector.tensor_copy(out=cnt_flat[:], in_=cnt_ps[:, 0:P])
                # transpose count row -> column via K=1 matmul
                cc_ps = ps_r.tile([P, 512], F32, tag="pr", name="cc_ps")
                nc.tensor.matmul(out=cc_ps[:, 0:1], lhsT=cnt_flat[0:1, :],
                                 rhs=ones_f[0:1, 0:1], start=True, stop=True)
                cnt_col = ph1.tile([P, 1], F32, tag="cnt_col")
                nc.vector.tensor_copy(out=cnt_col[:], in_=cc_ps[:, 0:1])
                # off_row[0, (i,e)] = sum_j cnt[(j,e)]*(j<i) + BASES[e]-1
                off_ps = ps_r.tile([1, 512], F32, tag="pr", name="off_ps")
                nc.tensor.matmul(out=off_ps[:, 0:P], lhsT=cnt_col[:, 0:1],
                                 rhs=pmask[:], start=True, stop=False)
                nc.tensor.matmul(out=off_ps[:, 0:P], lhsT=ones_f[0:1, 0:1],
                                 rhs=bases_row[:], start=False, stop=True)
                off_flat = ph1.tile([1, P], F32, tag="off_flat")
                nc.vector.tensor_copy(out=off_flat[:], in_=off_ps[:, 0:P])

                # ---- positions: within-tile inclusive prefix + offsets ----
                pos_ps = ps_r.tile([P, 512], F32, tag="pr", name="pos_ps")
                nc.tensor.matmul(out=pos_ps[:, 0:P], lhsT=u_incl[:], rhs=msk[:],
                                 start=True, stop=False)
                nc.tensor.matmul(out=pos_ps[:, 0:P], lhsT=ones_f[:, 0:P],
                                 rhs=off_flat[:], start=False, stop=True)

                tmp = ph1.tile([P, P], F32, tag="tmp")
                d1f = ph1.tile([P, NT], F32, tag="d1f")
                d2f = ph1.tile([P, NT], F32, tag="d2f")
                nc.vector.tensor_mul(out=tmp[:], in0=eq1[:], in1=pos_ps[:, 0:P])
                nc.vector.tensor_reduce(out=d1f[:], in_=r3(tmp), axis=AX.X,
                                        op=ALU.add)
                nc.vector.tensor_mul(out=tmp[:], in0=eq2[:], in1=pos_ps[:, 0:P])
                nc.vector.tensor_reduce(out=d2f[:], in_=r3(tmp), axis=AX.X,
                                        op=ALU.add)

                # ---- assemble scatter payload: dall [128,(s,t)] slot f32,
                #      wp_all [128,(s,t),2] = (w, tid) records ----
                # allocated from the long-lived const pool: the scatter ucode
                # reads these via SDMA after the instruction retires, so their
                # SBUF space must not be recycled by the FFN pools
                dall = ph1.tile([P, 2 * NT], F32, tag="dall")
                nc.vector.tensor_copy(out=dall[:, 0:NT], in_=d1f[:])
                nc.vector.tensor_copy(out=dall[:, NT:2 * NT], in_=d2f[:])
                wp_all = cpool.tile([P, 2 * NT * 2], F32, tag="wp_all")
                w4 = wp_all[:, :].rearrange("p (s t c) -> p s t c", s=2, c=2)
                wv3 = wv[:, :].rearrange("p (t s) -> p t s", s=2)
                nc.vector.tensor_copy(
                    out=w4[:, 0:1, :, 0:1],
                    in_=wv3[:, :, 0:1].unsqueeze(1))
                nc.vector.tensor_copy(
                    out=w4[:, 1:2, :, 0:1],
                    in_=wv3[:, :, 1:2].unsqueeze(1))
                nc.vector.tensor_copy(
                    out=w4[:, 0:1, :, 1:2],
                    in_=tid_col[:, :].unsqueeze(1).unsqueeze(3))
                nc.vector.tensor_copy(
                    out=w4[:, 1:2, :, 1:2],
                    in_=tid_col[:, :].unsqueeze(1).unsqueeze(3))

                # ---- wrapped-16 index staging for the scatter:
                # record q = r*128 + p  ->  idxs[q%16, q//16], i.e.
                # wrap[p%16, 8*r + p//16] = slot[p, r]. Built ON-CHIP with
                # PE transposes (one full transpose, then 8 [32,16]-block
                # transposes into stride-8 PSUM columns), cast to i16, ONE
                # contiguous DRAM write, and contiguous replicated reads —
                # replaces ~33us of element-granular DMA staging.
                dt_ps = ps_r.tile([P, 512], F32, tag="pr", name="dt_ps")
                nc.tensor.transpose(out=dt_ps[0:2 * NT, 0:P], in_=dall[:, :],
                                    identity=id_f32[:, :])
                dallT = ph1.tile([2 * NT, P], F32, tag="dallT")
                nc.vector.tensor_copy(out=dallT[:], in_=dt_ps[0:2 * NT, 0:P])
                # 8 block transposes [32,16]->[16,32]; each gets its own
                # PSUM bank (start=True zeroes the whole 2KB bank region):
                # reuse the 4 drained logits banks, two rounds
                idx16w = ph1.tile([16, NCOL], I16, tag="idx16w")
                i16v = idx16w[:, :].rearrange("p (r g) -> p r g", g=E)
                for g in range(E):
                    t2 = ps_lgt.tile([16, 512], F32, tag=f"lgt{g % 4}",
                                     name=f"t2_{g}")
                    nc.tensor.transpose(
                        out=t2[0:16, 0:2 * NT],
                        in_=dallT[:, 16 * g:16 * (g + 1)],
                        identity=id_f32[0:2 * NT, 0:2 * NT])
                    nc.vector.tensor_copy(
                        out=i16v[:, :, g:g + 1],
                        in_=t2[0:16, 0:2 * NT].unsqueeze(2))
                nc.sync.dma_start(out=pd2.ap()[:, :], in_=idx16w[:])
                idx_disp = cpool.tile([P, NCOL], I16, tag="idx_disp")
                for g in range(P // 16):
                    ring = nc.sync if g % 2 == 0 else nc.scalar
                    ring.dma_start(out=idx_disp[16 * g:16 * (g + 1), :],
                                   in_=pd2.ap()[:, :])
                nc.gpsimd.dma_scatter_add(
                    out_ap=wtbuf.ap()[:, 0:2],
                    in_ap=wp_all[:, :].rearrange("p (r c) -> p r c", c=2),
                    idxs_ap=idx_disp[:, :], num_idxs=2 * TLOC,
                    num_idxs_reg=2 * TLOC, elem_size=2,
                    elem_step=64).then_inc(disp_sem, 16)
                nc.gpsimd.wait_ge(disp_sem, 16)

            # ================= PHASE 2: expert FFN + combine ==============
            with tc.tile_pool(name="wpp", bufs=6) as wpp, \
                 tc.tile_pool(name="xet", bufs=2) as xetp, \
                 tc.tile_pool(name="h1t", bufs=1) as h1tp, \
                 tc.tile_pool(name="ysb", bufs=2) as ysbp, \
                 tc.tile_pool(name="wtk", bufs=2) as wtkp, \
                 tc.tile_pool(name="ps_s1", bufs=2, space="PSUM") as ps_s1, \
                 tc.tile_pool(name="ps_st", bufs=2, space="PSUM") as ps_st, \
                 tc.tile_pool(name="ps_y", bufs=1, space="PSUM") as ps_y:
                # the scatter ucode's DRAM write isn't tracked as a wtbuf
                # write by Tile: explicitly gate the wtbuf readers
                nc.sync.wait_ge(disp_sem, 16)
                nc.scalar.wait_ge(disp_sem, 16)

                def prefetch_disp(e, md_ring=nc.sync):
                    """Stage expert e's dispatch metadata and routed tokens.
                    The wrapped tid read is element-granular (~4us transfer),
                    so it runs ONCE into 16 partitions on the gpsimd ring;
                    the tiny i16 result is replicated to all 8 gpsimd-core
                    groups via a contiguous DRAM bounce. All reads here live
                    on gpsimd/scalar so a pending wait never blocks the
                    sync-ring proj weight stream (DMA queues are FIFO)."""
                    CAP, CAPR, BASE = CAPS[e], CAPRS[e], BASES[e]
                    NW = CAPR // 16
                    SUBS = _chunks(CAP, P)
                    tidw_f = wtkp.tile([P, NW], F32, tag="tidw_f",
                                       name=f"tidw_{e}")
                    nc.gpsimd.dma_start(
                        out=tidw_f[0:16, :],
                        in_=wtbuf.ap()[BASE:BASE + CAPR, 1:2].rearrange(
                            "(s p) o -> p (s o)", p=16))
                    idx16 = wtkp.tile([P, NW], I16, tag="idx16",
                                      name=f"idx16_{e}")
                    nc.vector.tensor_copy(out=idx16[0:16, :],
                                          in_=tidw_f[0:16, :])
                    nc.gpsimd.dma_start(out=pdx.ap()[e, :, 0:NW],
                                        in_=idx16[0:16, :])
                    idx = wtkp.tile([P, NW], I16, tag="idx",
                                    name=f"idx_{e}")
                    for g in range(P // 16):
                        md_ring.dma_start(out=idx[16 * g:16 * (g + 1), :],
                                          in_=pdx.ap()[e, :, 0:NW])
                    w_t = []
                    for ci, (cs, cw) in enumerate(SUBS):
                        wt = wtkp.tile([P, 1], F32, tag=f"wt{ci}",
                                       name=f"wt{ci}_{e}")
                        md_ring.dma_start(
                            out=wt[0:cw, :],
                            in_=wtbuf.ap()[BASE + cs:BASE + cs + cw, 0:1])
                        w_t.append(wt)
                    # gather x rows transposed:
                    # xet3d[p, k, c] = x[tid[c], k*128+p]
                    xet3d = xetp.tile([P, KH * CAPR], BF16, tag="xet3d",
                                      name=f"xet_{e}")
                    nc.gpsimd.dma_gather(
                        out_ap=xet3d[:, :].rearrange("p (k c) -> p k c", k=KH),
                        in_ap=xbf.ap()[:, :],
                        idxs_ap=idx[:, :], num_idxs=CAPR, num_idxs_reg=CAPR,
                        elem_size=H, transpose=True)
                    return idx, w_t, xet3d

                dstage = {EORDER[0]: prefetch_disp(EORDER[0],
                                                    md_ring=nc.scalar)}
                # re-ramp the PE p-state during the dispatch tail: wramp is
                # memset on the vector engine AFTER expert 0's idx cast, so
                # these warmups execute right before expert 0's fc
                wramp = cpool.tile([P, 512], BF16, tag="wramp")
                nc.vector.memset(wramp[:], 0.0)
                for wi in range(24):
                    wps2 = ps_s1.tile([P, 512], F32, tag="ps1",
                                      name=f"wps2_{wi}")
                    nc.tensor.matmul(out=wps2[:], lhsT=wramp[:, 0:P],
                                     rhs=wramp[:], start=True, stop=True)
                for ei, e in enumerate(EORDER):
                    CAP, CAPR, BASE = CAPS[e], CAPRS[e], BASES[e]
                    assert CAP <= 640
                    SUBS = _chunks(CAP, P)
                    NSUB = len(SUBS)
                    NSUBM = min(NSUB, 4)
                    TAILW = CAP - 512 if CAP > 512 else 0
                    wfc_k, bfc_sb, bpj_sb = wstage.pop(e)
                    idx, w_t, xet3d = dstage.pop(e)
                    # prefetch next expert's dispatch (gather overlaps fc)
                    if ei + 1 < E:
                        dstage[EORDER[ei + 1]] = prefetch_disp(EORDER[ei + 1])

                    # ---------------- fc: h1 = gelu(x @ Wfc) ----------------
                    # CAP <= 512: single 512-wide segment (one PSUM bank).
                    # CAP > 512: split at 256 so both segments are >= 107ns
                    # of PE rows (no LDWEIGHTS stall on tiny tail matmuls).
                    # Segment list: (col0, width, h1 tile, tile stride)
                    h1all = h1tp.tile([P, KF * 512], BF16, tag="h1all",
                                      name=f"h1_{e}")
                    h1tl = None
                    if TAILW:
                        h1tl = h1tp.tile([P, KF * 384], BF16, tag="h1tl",
                                         name=f"h1t_{e}")
                        FSEG = [(0, 256, h1all, 512, ps_s1),
                                (256, CAP - 256, h1tl, 384, ps_st)]
                    else:
                        FSEG = [(0, CAP, h1all, 512, ps_s1)]
                    for ft in range(KF):
                        pss = []
                        for si, (_, _, _, _, pool) in enumerate(FSEG):
                            ptag = "ps1" if pool is ps_s1 else "pst"
                            pss.append(pool.tile([P, 512], F32, tag=ptag,
                                                 name=f"pss{si}"))
                        for k in range(KH):
                            for si, (c0, cwid, _, _, _) in enumerate(FSEG):
                                nc.tensor.matmul(
                                    out=pss[si][:, 0:cwid],
                                    lhsT=wfc_k[k][:, ft * P:(ft + 1) * P],
                                    rhs=xet3d[:, k * CAPR + c0:
                                              k * CAPR + c0 + cwid],
                                    start=(k == 0), stop=(k == KH - 1))
                        bias_ap = (zbias[:] if zero_bias
                                   else bfc_sb[:, ft:ft + 1])
                        for si, (c0, cwid, h1t_, hstride, _) in \
                                enumerate(FSEG):
                            nc.scalar.activation(
                                out=h1t_[:, ft * hstride:ft * hstride + cwid],
                                in_=pss[si][:, 0:cwid],
                                func=_ACT_FN, bias=bias_ap)

                    def h1_slice(k, cs, cw):
                        """lhsT slice for proj sub [cs, cs+cw)."""
                        if TAILW and cs >= 256:
                            return h1tl[:, k * 384 + cs - 256:
                                        k * 384 + cs - 256 + cw]
                        return h1all[:, k * 512 + cs:k * 512 + cs + cw]

                    # fc weights for e are no longer needed: stream e+2's now
                    # (scalar ring; WAR on the last fc matmul read above)
                    if ei + 2 < E:
                        wstage[EORDER[ei + 2]] = prefetch_weights(
                            EORDER[ei + 2], nc.scalar)

                    # ------------- proj: y = w * (h1 @ Wproj) ---------------
                    ysb = ysbp.tile([P, NSUB * H], F32, tag="ysb",
                                    name=f"ysb_{e}")
                    if CAP % P:
                        # zero the partial last sub's block: its rows cw:128
                        # are scatter-added (to the pad token) and must be 0
                        nc.vector.memset(
                            ysb[0:P, (NSUB - 1) * H:NSUB * H], 0.0)
                    for hi, (hs, hw) in enumerate(HT):
                        psy = [ps_y.tile([P, 512], F32, tag=f"psy{ci}",
                                         name=f"psy{ci}{hi}_{e}")
                               for ci in range(NSUBM)]
                        ptl = None
                        if TAILW:
                            ptl = ps_st.tile([P, 512], F32, tag="pst",
                                             name=f"ptl{hi}_{e}")
                        for k2 in range(KF // 2):
                            # load TWO k-chunks per DMA: halves the issue
                            # count so the ring keeps pace with the PE in
                            # the 256-wide pass
                            wp = wpp.tile([P, 2 * 512], BF16, tag="wp")
                            nc.sync.dma_start(
                                out=wp[:, :].rearrange(
                                    "p (b c) -> p b c", b=2)[:, :, 0:hw],
                                in_=wpj.ap()[e, 2 * k2 * P:
                                             (2 * k2 + 2) * P,
                                             hs:hs + hw].rearrange(
                                    "(b p) c -> p b c", p=P))
                            for b in range(2):
                                k = 2 * k2 + b
                                wps = wp[:, b * 512:b * 512 + hw]
                                for ci in range(NSUBM):
                                    cs, cw = SUBS[ci]
                                    nc.tensor.matmul(
                                        out=psy[ci][0:cw, 0:hw],
                                        lhsT=h1_slice(k, cs, cw),
                                        rhs=wps,
                                        start=(k == 0),
                                        stop=(zero_bias and k == KF - 1))
                                if TAILW:
                                    cs, cw = SUBS[4]
                                    nc.tensor.matmul(
                                        out=ptl[0:cw, 0:hw],
                                        lhsT=h1_slice(k, cs, cw),
                                        rhs=wps,
                                        start=(k == 0),
                                        stop=(zero_bias and k == KF - 1))
                        for ci in range(NSUB):
                            cs, cw = SUBS[ci]
                            pt = ptl if ci == 4 else psy[ci]
                            if not zero_bias:
                                nc.tensor.matmul(
                                    out=pt[0:cw, 0:hw],
                                    lhsT=ones_bf[:, 0:cw],
                                    rhs=bpj_sb[:, hs:hs + hw],
                                    start=False, stop=True)
                            nc.vector.tensor_scalar_mul(
                                ysb[0:cw, ci * H + hs:ci * H + hs + hw],
                                pt[0:cw, 0:hw],
                                w_t[ci][0:cw, 0:1])
                    # one scatter-add per expert: per-sub ucode calls
                    # serialize at ~5.7us each on the gpsimd queue
                    nc.gpsimd.dma_scatter_add(
                        out_ap=out.ap()[:, :],
                        in_ap=ysb[:, :].rearrange("p (n h) -> p n h", n=NSUB),
                        idxs_ap=idx[:, :], num_idxs=CAPR, num_idxs_reg=CAPR,
                        elem_size=H)

    nc.compile()
    return nc


# ---------------------------------------------------------------------------
_BUILD_CACHE = {}
_LAST_IN_MAPS = None


def _get_built(TLOC, H, F, E, CAPS, zero_bias=False):
    key = (TLOC, H, F, E, tuple(CAPS), zero_bias)
    if key not in _BUILD_CACHE:
        _BUILD_CACHE[key] = build_moe(TLOC, H, F, E, tuple(CAPS),
                                      zero_bias=zero_bias)
    return _BUILD_CACHE[key]


def kernel(hidden_states, Wr, br, Wfc, bfc, Wproj, bproj):
    from concourse.bass_utils import run_bass_kernel_spmd

    hs = np.ascontiguousarray(np.asarray(hidden_states, dtype=np.float32))
    Wr = np.ascontiguousarray(np.asarray(Wr, dtype=np.float32))
    br = np.ascontiguousarray(np.asarray(br, dtype=np.float32))
    Wfc = np.asarray(Wfc, dtype=np.float32)
    bfc = np.ascontiguousarray(np.asarray(bfc, dtype=np.float32))
    Wproj = np.asarray(Wproj, dtype=np.float32)
    bproj = np.asarray(bproj, dtype=np.float32)

    B, S, H = hs.shape
    E, H2, F = Wfc.shape
    assert H2 == H
    T = B * S
    assert T % N_CORES == 0
    TLOC = T // N_CORES
    x = hs.reshape(T, H)

    # host-side routing peek ONLY to pick compile-time capacities and the
    # balanced token->core assignment (routing itself runs on device)
    logits = x @ Wr + br[None, :]
    top2 = np.argpartition(-logits, 2, axis=1)[:, :2]
    pair_lo = np.minimum(top2[:, 0], top2[:, 1])
    pair_hi = np.maximum(top2[:, 0], top2[:, 1])
    type_id = pair_lo.astype(np.int64) * E + pair_hi
    order = np.argsort(type_id, kind="stable")
    # deal the type-sorted token list round-robin: core g takes order[g::8]
    core_toks = [order[g::N_CORES] for g in range(N_CORES)]
    maxcnt = np.zeros(E, dtype=np.int64)
    for g in range(N_CORES):
        cnts = np.bincount(top2[core_toks[g]].ravel(), minlength=E)
        maxcnt = np.maximum(maxcnt, cnts)
    CAPS = tuple(int(max(128, math.ceil((m + 4) / 8.0) * 8))
                 for m in maxcnt)

    zero_bias = bool(np.all(bfc == 0.0) and np.all(bproj == 0.0)
                     and np.all(br == 0.0))
    nc = _get_built(TLOC, H, F, E, CAPS, zero_bias)

    wfc_bf = np.ascontiguousarray(Wfc.astype(ml_dtypes.bfloat16))
    wpj_bf = np.ascontiguousarray(Wproj.astype(ml_dtypes.bfloat16))

    in_maps = []
    for g in range(N_CORES):
        xc = x[core_toks[g]]
        xbf_c = np.zeros((TLOC + 128, H), dtype=ml_dtypes.bfloat16)
        xbf_c[:TLOC] = xc.astype(ml_dtypes.bfloat16)
        m = {
            "xt": np.ascontiguousarray(xc.T),
            "xbf": xbf_c,
            "wr": Wr,
            "wfc": wfc_bf,
            "wpj": wpj_bf,
        }
        if not zero_bias:
            m["brr"] = np.ascontiguousarray(br.reshape(1, E))
            m["bfc"] = bfc
            m["bpj"] = np.ascontiguousarray(bproj.astype(ml_dtypes.bfloat16))
        in_maps.append(m)

    global _LAST_IN_MAPS
    _LAST_IN_MAPS = in_maps

    res = run_bass_kernel_spmd(nc, in_maps, core_ids=list(range(N_CORES)))
    full = np.empty((T, H), dtype=np.float32)
    for g in range(N_CORES):
        full[core_toks[g]] = res.results[g]["out"][:TLOC]
    return full.reshape(B, S, H).astype(np.float32)


# revision 49
# speedup vs baseline: 1.2243x; 1.0037x over previous
"""MoE (8 experts, top-2) Bass kernel for 8 trn2 NeuronCores.

Strategy: data-parallel over tokens with HOST-BALANCED token->core assignment.
The host groups tokens by their top-2 expert pair (type) and deals them
round-robin across the 8 cores, so every core sees ~C_e/8 tokens per expert.
That pins the per-expert capacity CAP_e at ~512, which removes the padded
5th proj sub-tile and the fc tail matmuls that dominate PE waste when
capacities are unbalanced.

On device, per core:
  phase 1: router logits via fp32 matmuls on host-transposed x chunks
           (per-k-chunk DMAs pipelined with PE warmup), batched top-2 /
           combine-weight math, counts -> offsets via an on-chip prefix
           matmul (constant prefix mask; no DRAM round-trips), positions,
           then ONE (w, token_id) record scatter into the position-ordered
           dispatch table. The scatter's wrapped-16 index layout is staged
           through DRAM with 8 small strided writes spread over all three
           DMA rings + contiguous replicated readbacks, replacing the
           baseline's 8 full strided re-reads (~70us -> ~15us).
  phase 2: per expert: dma_gather(transpose=True) routed token rows into
           [H-chunk, c] matmul layout, dense FFN gelu(x@Wfc)@Wproj in bf16
           with fp32 accumulate (fc split at 256 for over-512 capacities so
           no matmul is LDWEIGHTS-bound), scale rows by combine weight, one
           dma_scatter_add per expert. Ring discipline (DMA queues are FIFO
           and a waiting dma_start blocks its ENGINE): sync carries only the
           paired proj-weight stream + expert metadata whose waits have
           slack; scalar carries activations + fc weight prefetch (2 experts
           ahead; during phase 1 on the idle gpsimd ring); gpsimd carries
           zero-init, the wrapped tid read (element-granular), gathers and
           scatters. The wrapped gather indices are read once into 16
           partitions, cast to i16, and replicated to all 8 gpsimd-core
           groups via a tiny contiguous DRAM bounce. PE p-state is re-ramped
           with dependency-gated warmup matmuls right before expert 0.

Host does only slicing/concat/layout staging (permutation + transpose +
bf16 cast) plus a routing peek to pick compile-time capacities and the
balanced assignment.
"""

import math
import os
import sys

import numpy as np

for _p in ("/opt/trn_rl_repo", "/root/.axon_site/_ro/trn_rl_repo"):
    if os.path.isdir(_p) and _p not in sys.path:
        sys.path.insert(0, _p)

import ml_dtypes  # noqa: E402
import concourse.bass as bass  # noqa: E402
import concourse.mybir as mybir  # noqa: E402
import concourse.tile as tile  # noqa: E402
from concourse import bacc  # noqa: E402
from concourse.masks import make_upper_triangular, make_identity  # noqa: E402
from concourse import library_config  # noqa: E402

F32 = mybir.dt.float32
BF16 = mybir.dt.bfloat16
I32 = mybir.dt.int32
I16 = mybir.dt.int16
AF = mybir.ActivationFunctionType
ALU = mybir.AluOpType
AX = mybir.AxisListType

N_CORES = 8
P = 128
_ACT_FN = AF.Gelu_apprx_tanh  # debug hook: sim lacks gelu, tests swap in Tanh


def _chunks(total, step):
    out = []
    off = 0
    while off < total:
        w = min(step, total - off)
        out.append((off, w))
        off += w
    return out


def build_moe(TLOC, H, F, E, CAPS, zero_bias=False):
    """Build the per-core Bass program (SPMD: identical on all cores)."""
    assert TLOC % P == 0 and H % P == 0 and F % P == 0 and E == 8
    CAPS = list(CAPS)
    assert len(CAPS) == E and all(c % 8 == 0 for c in CAPS)
    CAPRS = [((c + P - 1) // P) * P for c in CAPS]
    KH = H // P            # contraction chunks over H (6)
    KF = F // P            # f-tiles (and stage-2 contraction chunks) (24)
    NT = TLOC // P         # token tiles (16)
    assert NT == 16
    BASES = [sum(CAPRS[:e]) for e in range(E)]
    NPOS = sum(CAPRS) + P
    NPOS = ((NPOS + P - 1) // P) * P
    NBLK = NPOS // P
    HT = ((0, 512), (512, H - 512))  # proj h-tiles

    nc = bacc.Bacc("TRN2", target_bir_lowering=False, debug=False,
                   enable_asserts=True, num_devices=N_CORES)

    xt = nc.dram_tensor("xt", [H, TLOC], F32, kind="ExternalInput")
    xbf = nc.dram_tensor("xbf", [TLOC + P, H], BF16, kind="ExternalInput")
    wr = nc.dram_tensor("wr", [H, E], F32, kind="ExternalInput")
    wfc = nc.dram_tensor("wfc", [E, H, F], BF16, kind="ExternalInput")
    wpj = nc.dram_tensor("wpj", [E, F, H], BF16, kind="ExternalInput")
    if not zero_bias:
        brr = nc.dram_tensor("brr", [1, E], F32, kind="ExternalInput")
        bfc = nc.dram_tensor("bfc", [E, F], F32, kind="ExternalInput")
        bpj = nc.dram_tensor("bpj", [E, H], BF16, kind="ExternalInput")
    # extra trash tile rows at the end absorb pad-slot scatter-adds
    out = nc.dram_tensor("out", [TLOC + P, H], F32, kind="ExternalOutput")

    # dispatch table: (combine w, token id) in cols 0:2 of 256B-strided rows
    # (dma_scatter_add needs a 256B row stride)
    wtbuf = nc.dram_tensor("wtbuf", [NPOS, 64], F32)
    # wrapped-16 dispatch index staging: PD2[p%16, 8*r + p//16] = dsi[p, r]
    NCOL = 2 * TLOC // 16
    pd2 = nc.dram_tensor("pd2", [16, NCOL], I16)
    # per-expert gather-index replication bounce (i16, wrap-16 layout)
    MAXW = max(CAPRS) // 16
    pdx = nc.dram_tensor("pdx", [E, 16, MAXW], I16)

    with tile.TileContext(nc) as tc:
        with tc.tile_pool(name="const", bufs=1) as cpool, \
             tc.tile_pool(name="wc", bufs=2) as wc:
            # ---------------- constants ----------------
            u_incl = cpool.tile([P, P], F32, tag="u_incl")
            make_upper_triangular(nc, u_incl, val=1.0, diag=True)
            ones_f = cpool.tile([1, 512], F32, tag="ones_f")
            nc.gpsimd.memset(ones_f[:], 1.0)
            ones_col = cpool.tile([P, 1], F32, tag="ones_col")
            nc.gpsimd.memset(ones_col[:], 1.0)
            zbias = cpool.tile([P, 1], F32, tag="zbias")
            nc.gpsimd.memset(zbias[:], 0.0)
            id_f32 = cpool.tile([P, P], F32, tag="id_f32")
            make_identity(nc, id_f32)
            # token ids: tid_col[p, c] = p + 128*c
            tid_i = cpool.tile([P, NT], I32, tag="tid_i")
            nc.gpsimd.iota(tid_i[:], pattern=[[P, NT]], base=0,
                           channel_multiplier=1)
            tid_col = cpool.tile([P, NT], F32, tag="tid_col")
            nc.vector.tensor_copy(out=tid_col[:], in_=tid_i[:])
            zbig = cpool.tile([P, H], F32, tag="zbig")
            nc.gpsimd.memset(zbig[:], 0.0)
            wzero = cpool.tile([P, 512], BF16, tag="wzero")
            nc.gpsimd.memset(wzero[:], 0.0)
            ones_bf = cpool.tile([1, P], BF16, tag="ones_bf")
            nc.gpsimd.memset(ones_bf[:], 1.0)

            # --- constants for the on-chip count->offset prefix ---
            # flattening: column/partition index (t, e) -> t*8 + e
            # pmask[p=(j,e'), m=(i,e)] = (j < i) * (e' == e)
            tm_i = cpool.tile([P, P], I32, tag="tm_i")
            nc.gpsimd.iota(tm_i[:].rearrange("p (t e) -> p t e", e=E),
                           pattern=[[1, NT], [0, E]], base=0,
                           channel_multiplier=0)
            tm_row = cpool.tile([P, P], F32, tag="tm_row")
            nc.vector.tensor_copy(out=tm_row[:], in_=tm_i[:])
            em_i = cpool.tile([P, P], I32, tag="em_i")
            nc.gpsimd.iota(em_i[:].rearrange("p (t e) -> p t e", e=E),
                           pattern=[[0, NT], [1, E]], base=0,
                           channel_multiplier=0)
            em_row = cpool.tile([P, P], F32, tag="em_row")
            nc.vector.tensor_copy(out=em_row[:], in_=em_i[:])
            # tp_col/ep_col (per-partition t/e of index p) and pmask are
            # built inside phase 1: they need a PSUM pool for the K=1
            # transpose matmuls (engine memsets can't target partition
            # offsets that aren't 32-aligned)
            pmask = cpool.tile([P, P], F32, tag="pmask")
            # bases_row[0, (t,e)] = BASES[e] - 1
            bases_row = cpool.tile([1, P], F32, tag="bases_row")
            for e in range(E):
                nc.gpsimd.memset(
                    bases_row[:].rearrange("p (t e) -> p t e", e=E)
                    [:, :, e:e + 1], float(BASES[e] - 1))

            # dma_gather / dma_scatter_add ucode lives in the mlp library.
            nc.gpsimd.load_library(library_config.mlp)


            disp_sem = nc.alloc_semaphore("disp_dma")

            EORDER = sorted(range(E), key=lambda ee: -CAPS[ee])

            def prefetch_weights(e, ring):
                """Stage expert e's fc weights (+biases). Phase 1 uses the
                gpsimd ring (idle then); phase 2 the scalar ring (only
                activations there), keeping sync free for proj weights."""
                wfc_k = []
                for k in range(KH):
                    wk = wc.tile([P, F], BF16, tag=f"wfc{k}",
                                 name=f"wfc{k}_{e}")
                    ring.dma_start(
                        out=wk[:], in_=wfc.ap()[e, k * P:(k + 1) * P, :])
                    wfc_k.append(wk)
                bfc_sb = None
                bpj_sb = None
                if not zero_bias:
                    bfc_sb = wc.tile([P, KF], F32, tag="bfc_sb",
                                     name=f"bfc_{e}")
                    ring.dma_start(
                        out=bfc_sb[:],
                        in_=bfc.ap()[e:e + 1, :].rearrange(
                            "o (a p) -> p (o a)", p=P))
                    bpj_sb = wc.tile([1, H], BF16, tag="bpj_sb",
                                     name=f"bpj_{e}")
                    ring.dma_start(out=bpj_sb[:],
                                   in_=bpj.ap()[e:e + 1, :])
                return wfc_k, bfc_sb, bpj_sb

            # ============ PHASE 1: router + dispatch (batched) ============
            with tc.tile_pool(name="ph1", bufs=1) as ph1, \
                 tc.tile_pool(name="xtk", bufs=1) as xtkp, \
                 tc.tile_pool(name="ps_wu", bufs=2, space="PSUM") as ps_wu, \
                 tc.tile_pool(name="ps_lgt", bufs=1, space="PSUM") as ps_lgt, \
                 tc.tile_pool(name="ps_r", bufs=1, space="PSUM") as ps_r:
                # router inputs first on the sync ring (latency-critical)
                wr_sb = ph1.tile([P, KH * E], F32, tag="wr_sb")
                nc.sync.dma_start(
                    out=wr_sb[:, :].rearrange("p (k e) -> p k e", e=E),
                    in_=wr.ap()[:, :].rearrange("(k p) e -> p k e", p=P))
                # x chunks: one DMA per k so logits pipeline with the load
                xks = []
                for k in range(KH):
                    xk = xtkp.tile([P, TLOC], F32, tag=f"xk{k}")
                    ring = nc.sync if k % 2 == 0 else nc.scalar
                    ring.dma_start(
                        out=xk[:], in_=xt.ap()[k * P:(k + 1) * P, :])
                    xks.append(xk)
                if not zero_bias:
                    br_sb = ph1.tile([1, E], F32, tag="br_sb")
                    nc.sync.dma_start(out=br_sb[:], in_=brr.ap()[:, :])

                # ---- build pmask[p=(j,e'), m=(i,e)] = (j<i)*(e'==e):
                # transpose row 0 of the iota constants to per-partition
                # columns via K=1 matmuls, then two compares + a multiply
                pc_ps = ps_r.tile([P, 512], F32, tag="pr", name="pc_ps")
                nc.tensor.matmul(out=pc_ps[:, 0:1], lhsT=tm_row[0:1, :],
                                 rhs=ones_f[0:1, 0:1], start=True, stop=True)
                nc.tensor.matmul(out=pc_ps[:, 1:2], lhsT=em_row[0:1, :],
                                 rhs=ones_f[0:1, 0:1], start=True, stop=True)
                tp_col = ph1.tile([P, 2], F32, tag="tp_col")
                nc.vector.tensor_copy(out=tp_col[:], in_=pc_ps[:, 0:2])
                ptmp = ph1.tile([P, P], F32, tag="ptmp")
                nc.vector.tensor_scalar(out=pmask[:], in0=tm_row[:],
                                        scalar1=tp_col[:, 0:1], scalar2=None,
                                        op0=ALU.is_gt)
                nc.vector.tensor_scalar(out=ptmp[:], in0=em_row[:],
                                        scalar1=tp_col[:, 1:2], scalar2=None,
                                        op0=ALU.is_equal)
                nc.vector.tensor_mul(out=pmask[:], in0=pmask[:], in1=ptmp[:])

                def warm(n):
                    # keep the PE p-state ramped while inputs stream
                    for _ in range(n):
                        wps = ps_wu.tile([P, 512], F32, tag="wps")
                        nc.tensor.matmul(out=wps[:], lhsT=wzero[:, 0:P],
                                         rhs=wzero[:], start=True, stop=True)

                # init dispatch table + zero out (gpsimd ring: it is idle
                # during phase 1, and keeps sync/scalar free for the
                # dispatch critical path; WAW deps order these before the
                # scatter / scatter_add). Emitted BEFORE the weight
                # prefetches so the scatter isn't stuck behind them.
                for a0, aw in _chunks(NBLK, H // 64):
                    nc.gpsimd.dma_start(
                        out=wtbuf.ap()[:, :].rearrange(
                            "(a p) c -> p a c", p=P)[:, a0:a0 + aw, :],
                        in_=zbig[:, 0:aw * 64].rearrange(
                            "p (a c) -> p a c", c=64))

                # first two experts' fc weights don't depend on routing.
                # They go BEFORE the big output zero-fill on the gpsimd
                # queue so expert 0's gather/metadata transfers aren't
                # stuck behind 6.3MB of zeros after the dispatch scatter.
                wstage = {EORDER[0]: prefetch_weights(EORDER[0], nc.gpsimd),
                          EORDER[1]: prefetch_weights(EORDER[1], nc.gpsimd)}
                for i in range(NT + 1):
                    nc.gpsimd.dma_start(out=out.ap()[i * P:(i + 1) * P, :],
                                        in_=zbig[:])

                # ---- logits, Wr-stationary: logitsT [8, 2048] accumulated
                # k-major in 4 segment banks, warmups interleaved per k-group
                # to absorb the per-chunk DMA cadence, then PE-transposed ----
                # [16, 512] so the banks can be reused by the wrap-staging
                # block transposes after the logits drain (logits use rows
                # 0:8 only)
                lgt_ps = [ps_lgt.tile([16, 512], F32, tag=f"lgt{s}",
                                      name=f"lgt{s}") for s in range(4)]
                warm(18)
                for k in range(KH):
                    for s in range(4):
                        nc.tensor.matmul(
                            out=lgt_ps[s][0:E, :],
                            lhsT=wr_sb[:, k * E:(k + 1) * E],
                            rhs=xks[k][:, s * 512:(s + 1) * 512],
                            start=(k == 0),
                            stop=(k == KH - 1 and zero_bias))
                    if k < KH - 1:
                        warm(8)
                if not zero_bias:
                    for s in range(4):
                        nc.tensor.matmul(
                            out=lgt_ps[s][0:E, :], lhsT=br_sb[0:1, :],
                            rhs=ones_f[0:1, 0:512],
                            start=False, stop=True)
                lgt_sb = ph1.tile([E, TLOC], F32, tag="lgt_sb")
                for s in range(4):
                    if s % 2 == 0:
                        nc.vector.tensor_copy(
                            out=lgt_sb[:, s * 512:(s + 1) * 512],
                            in_=lgt_ps[s][0:E, :])
                    else:
                        nc.scalar.copy(
                            out=lgt_sb[:, s * 512:(s + 1) * 512],
                            in_=lgt_ps[s][0:E, :])
                lg_ps = ps_r.tile([P, 512], F32, tag="pr", name="lg_ps")
                for i in range(NT):
                    nc.tensor.transpose(
                        out=lg_ps[:, i * E:(i + 1) * E],
                        in_=lgt_sb[0:E, i * P:(i + 1) * P],
                        identity=id_f32[0:E, 0:E])
                lg = ph1.tile([P, P], F32, tag="lg")
                nc.vector.tensor_copy(out=lg[:], in_=lg_ps[:, 0:P])

                def r3(t):  # [128, (16,8)] -> [128, 16, 8]
                    return t[:, :].rearrange("p (t e) -> p t e", e=E)

                # ---- top-2 (per tile), then batched compare/combine ----
                m8 = ph1.tile([P, P], F32, tag="m8")
                for i in range(NT):
                    nc.vector.max(out=m8[:, i * E:(i + 1) * E],
                                  in_=lg[:, i * E:(i + 1) * E])
                eq1 = ph1.tile([P, P], F32, tag="eq1")
                eq2 = ph1.tile([P, P], F32, tag="eq2")
                msk = ph1.tile([P, P], F32, tag="msk")
                nc.vector.tensor_tensor(
                    out=r3(eq1), in0=r3(lg),
                    in1=r3(m8)[:, :, 0:1].to_broadcast([P, NT, E]),
                    op=ALU.is_equal)
                nc.vector.tensor_tensor(
                    out=r3(eq2), in0=r3(lg),
                    in1=r3(m8)[:, :, 1:2].to_broadcast([P, NT, E]),
                    op=ALU.is_equal)
                nc.vector.tensor_add(out=msk[:], in0=eq1[:], in1=eq2[:])

                # combine weights: w1 = sigmoid(m1-m2), w2 = sigmoid(m2-m1)
                dt_ = ph1.tile([P, 2 * NT], F32, tag="dt_")
                d3 = dt_[:, :].rearrange("p (t s) -> p t s", s=2)
                nc.vector.tensor_sub(out=d3[:, :, 0:1],
                                     in0=r3(m8)[:, :, 0:1],
                                     in1=r3(m8)[:, :, 1:2])
                nc.vector.tensor_sub(out=d3[:, :, 1:2],
                                     in0=r3(m8)[:, :, 1:2],
                                     in1=r3(m8)[:, :, 0:1])
                wv = ph1.tile([P, 2 * NT], F32, tag="wv")
                nc.scalar.activation(out=wv[:], in_=dt_[:], func=AF.Sigmoid,
                                     bias=zbias[:])

                # ---- counts -> offsets, fully on-chip (no DRAM bounce) ----
                cnt_ps = ps_r.tile([1, 512], F32, tag="pr", name="cnt_ps")
                nc.tensor.matmul(out=cnt_ps[:, 0:P], lhsT=ones_col[:, 0:1],
                                 rhs=msk[:], start=True, stop=True)
                cnt_flat = ph1.tile([1, P], F32, tag="cnt_flat")
                nc.vector.tensor_copy(out=cnt_flat[:], in_=cnt_ps[:, 0:P])
                # transpose count row -> column via K=1 matmul
                cc_ps = ps_r.tile([P, 512], F32, tag="pr", name="cc_ps")
                nc.tensor.matmul(out=cc_ps[:, 0:1], lhsT=cnt_flat[0:1, :],
                                 rhs=ones_f[0:1, 0:1], start=True, stop=True)
                cnt_col = ph1.tile([P, 1], F32, tag="cnt_col")
                nc.vector.tensor_copy(out=cnt_col[:], in_=cc_ps[:, 0:1])
                # off_row[0, (i,e)] = sum_j cnt[(j,e)]*(j<i) + BASES[e]-1
                off_ps = ps_r.tile([1, 512], F32, tag="pr", name="off_ps")
                nc.tensor.matmul(out=off_ps[:, 0:P], lhsT=cnt_col[:, 0:1],
                                 rhs=pmask[:], start=True, stop=False)
                nc.tensor.matmul(out=off_ps[:, 0:P], lhsT=ones_f[0:1, 0:1],
                                 rhs=bases_row[:], start=False, stop=True)
                off_flat = ph1.tile([1, P], F32, tag="off_flat")
                nc.vector.tensor_copy(out=off_flat[:], in_=off_ps[:, 0:P])

                # ---- positions: within-tile inclusive prefix + offsets ----
                pos_ps = ps_r.tile([P, 512], F32, tag="pr", name="pos_ps")
                nc.tensor.matmul(out=pos_ps[:, 0:P], lhsT=u_incl[:], rhs=msk[:],
                                 start=True, stop=False)
                nc.tensor.matmul(out=pos_ps[:, 0:P], lhsT=ones_f[:, 0:P],
                                 rhs=off_flat[:], start=False, stop=True)

                tmp = ph1.tile([P, P], F32, tag="tmp")
                d1f = ph1.tile([P, NT], F32, tag="d1f")
                d2f = ph1.tile([P, NT], F32, tag="d2f")
                nc.vector.tensor_mul(out=tmp[:], in0=eq1[:], in1=pos_ps[:, 0:P])
                nc.vector.tensor_reduce(out=d1f[:], in_=r3(tmp), axis=AX.X,
                                        op=ALU.add)
                nc.vector.tensor_mul(out=tmp[:], in0=eq2[:], in1=pos_ps[:, 0:P])
                nc.vector.tensor_reduce(out=d2f[:], in_=r3(tmp), axis=AX.X,
                                        op=ALU.add)

                # ---- assemble scatter payload: dall [128,(s,t)] slot f32,
                #      wp_all [128,(s,t),2] = (w, tid) records ----
                # allocated from the long-lived const pool: the scatter ucode
                # reads these via SDMA after the instruction retires, so their
                # SBUF space must not be recycled by the FFN pools
                dall = ph1.tile([P, 2 * NT], F32, tag="dall")
                nc.vector.tensor_copy(out=dall[:, 0:NT], in_=d1f[:])
                nc.vector.tensor_copy(out=dall[:, NT:2 * NT], in_=d2f[:])
                wp_all = cpool.tile([P, 2 * NT * 2], F32, tag="wp_all")
                w4 = wp_all[:, :].rearrange("p (s t c) -> p s t c", s=2, c=2)
                wv3 = wv[:, :].rearrange("p (t s) -> p t s", s=2)
                nc.vector.tensor_copy(
                    out=w4[:, 0:1, :, 0:1],
                    in_=wv3[:, :, 0:1].unsqueeze(1))
                nc.vector.tensor_copy(
                    out=w4[:, 1:2, :, 0:1],
                    in_=wv3[:, :, 1:2].unsqueeze(1))
                nc.vector.tensor_copy(
                    out=w4[:, 0:1, :, 1:2],
                    in_=tid_col[:, :].unsqueeze(1).unsqueeze(3))
                nc.vector.tensor_copy(
                    out=w4[:, 1:2, :, 1:2],
                    in_=tid_col[:, :].unsqueeze(1).unsqueeze(3))

                # ---- wrapped-16 index staging for the scatter:
                # record q = r*128 + p  ->  idxs[q%16, q//16], i.e.
                # wrap[p%16, 8*r + p//16] = slot[p, r]. Built ON-CHIP with
                # PE transposes (one full transpose, then 8 [32,16]-block
                # transposes into stride-8 PSUM columns), cast to i16, ONE
                # contiguous DRAM write, and contiguous replicated reads —
                # replaces ~33us of element-granular DMA staging.
                dt_ps = ps_r.tile([P, 512], F32, tag="pr", name="dt_ps")
                nc.tensor.transpose(out=dt_ps[0:2 * NT, 0:P], in_=dall[:, :],
                                    identity=id_f32[:, :])
                dallT = ph1.tile([2 * NT, P], F32, tag="dallT")
                nc.vector.tensor_copy(out=dallT[:], in_=dt_ps[0:2 * NT, 0:P])
                # 8 block transposes [32,16]->[16,32]; each gets its own
                # PSUM bank (start=True zeroes the whole 2KB bank region):
                # reuse the 4 drained logits banks, two rounds
                idx16w = ph1.tile([16, NCOL], I16, tag="idx16w")
                i16v = idx16w[:, :].rearrange("p (r g) -> p r g", g=E)
                for g in range(E):
                    t2 = ps_lgt.tile([16, 512], F32, tag=f"lgt{g % 4}",
                                     name=f"t2_{g}")
                    nc.tensor.transpose(
                        out=t2[0:16, 0:2 * NT],
                        in_=dallT[:, 16 * g:16 * (g + 1)],
                        identity=id_f32[0:2 * NT, 0:2 * NT])
                    nc.vector.tensor_copy(
                        out=i16v[:, :, g:g + 1],
                        in_=t2[0:16, 0:2 * NT].unsqueeze(2))
                nc.sync.dma_start(out=pd2.ap()[:, :], in_=idx16w[:])
                idx_disp = cpool.tile([P, NCOL], I16, tag="idx_disp")
                for g in range(P // 16):
                    ring = nc.sync if g % 2 == 0 else nc.scalar
                    ring.dma_start(out=idx_disp[16 * g:16 * (g + 1), :],
                                   in_=pd2.ap()[:, :])
                nc.gpsimd.dma_scatter_add(
                    out_ap=wtbuf.ap()[:, 0:2],
                    in_ap=wp_all[:, :].rearrange("p (r c) -> p r c", c=2),
                    idxs_ap=idx_disp[:, :], num_idxs=2 * TLOC,
                    num_idxs_reg=2 * TLOC, elem_size=2,
                    elem_step=64).then_inc(disp_sem, 16)
                nc.gpsimd.wait_ge(disp_sem, 16)

            # ================= PHASE 2: expert FFN + combine ==============
            with tc.tile_pool(name="wpp", bufs=6) as wpp, \
                 tc.tile_pool(name="xet", bufs=2) as xetp, \
                 tc.tile_pool(name="h1t", bufs=1) as h1tp, \
                 tc.tile_pool(name="ysb", bufs=2) as ysbp, \
                 tc.tile_pool(name="wtk", bufs=2) as wtkp, \
                 tc.tile_pool(name="ps_s1", bufs=2, space="PSUM") as ps_s1, \
                 tc.tile_pool(name="ps_st", bufs=2, space="PSUM") as ps_st, \
                 tc.tile_pool(name="ps_y", bufs=1, space="PSUM") as ps_y:
                # the scatter ucode's DRAM write isn't tracked as a wtbuf
                # write by Tile: explicitly gate the wtbuf readers
                nc.sync.wait_ge(disp_sem, 16)
                nc.scalar.wait_ge(disp_sem, 16)

                def prefetch_disp(e, md_ring=nc.sync):
                    """Stage expert e's dispatch metadata and routed tokens.
                    The wrapped tid read is element-granular (~4us transfer),
                    so it runs ONCE into 16 partitions on the gpsimd ring;
                    the tiny i16 result is replicated to all 8 gpsimd-core
                    groups via a contiguous DRAM bounce. All reads here live
                    on gpsimd/scalar so a pending wait never blocks the
                    sync-ring proj weight stream (DMA queues are FIFO)."""
                    CAP, CAPR, BASE = CAPS[e], CAPRS[e], BASES[e]
                    NW = CAPR // 16
                    SUBS = _chunks(CAP, P)
                    tidw_f = wtkp.tile([P, NW], F32, tag="tidw_f",
                                       name=f"tidw_{e}")
                    nc.gpsimd.dma_start(
                        out=tidw_f[0:16, :],
                        in_=wtbuf.ap()[BASE:BASE + CAPR, 1:2].rearrange(
                            "(s p) o -> p (s o)", p=16))
                    idx16 = wtkp.tile([P, NW], I16, tag="idx16",
                                      name=f"idx16_{e}")
                    nc.vector.tensor_copy(out=idx16[0:16, :],
                                          in_=tidw_f[0:16, :])
                    nc.gpsimd.dma_start(out=pdx.ap()[e, :, 0:NW],
                                        in_=idx16[0:16, :])
                    idx = wtkp.tile([P, NW], I16, tag="idx",
                                    name=f"idx_{e}")
                    for g in range(P // 16):
                        md_ring.dma_start(out=idx[16 * g:16 * (g + 1), :],
                                          in_=pdx.ap()[e, :, 0:NW])
                    w_t = []
                    for ci, (cs, cw) in enumerate(SUBS):
                        wt = wtkp.tile([P, 1], F32, tag=f"wt{ci}",
                                       name=f"wt{ci}_{e}")
                        md_ring.dma_start(
                            out=wt[0:cw, :],
                            in_=wtbuf.ap()[BASE + cs:BASE + cs + cw, 0:1])
                        w_t.append(wt)
                    # gather x rows transposed:
                    # xet3d[p, k, c] = x[tid[c], k*128+p]
                    xet3d = xetp.tile([P, KH * CAPR], BF16, tag="xet3d",
                                      name=f"xet_{e}")
                    nc.gpsimd.dma_gather(
                        out_ap=xet3d[:, :].rearrange("p (k c) -> p k c", k=KH),
                        in_ap=xbf.ap()[:, :],
                        idxs_ap=idx[:, :], num_idxs=CAPR, num_idxs_reg=CAPR,
                        elem_size=H, transpose=True)
                    return idx, w_t, xet3d

                dstage = {EORDER[0]: prefetch_disp(EORDER[0],
                                                    md_ring=nc.scalar)}
                # re-ramp the PE p-state during the dispatch tail: wramp is
                # memset on the vector engine AFTER expert 0's idx cast, so
                # these warmups execute right before expert 0's fc
                wramp = cpool.tile([P, 512], BF16, tag="wramp")
                nc.vector.memset(wramp[:], 0.0)
                for wi in range(24):
                    wps2 = ps_s1.tile([P, 512], F32, tag="ps1",
                                      name=f"wps2_{wi}")
                    nc.tensor.matmul(out=wps2[:], lhsT=wramp[:, 0:P],
                                     rhs=wramp[:], start=True, stop=True)
                for ei, e in enumerate(EORDER):
                    CAP, CAPR, BASE = CAPS[e], CAPRS[e], BASES[e]
                    assert CAP <= 640
                    SUBS = _chunks(CAP, P)
                    NSUB = len(SUBS)
                    NSUBM = min(NSUB, 4)
                    TAILW = CAP - 512 if CAP > 512 else 0
                    wfc_k, bfc_sb, bpj_sb = wstage.pop(e)
                    idx, w_t, xet3d = dstage.pop(e)
                    # prefetch next expert's dispatch (gather overlaps fc)
                    if ei + 1 < E:
                        dstage[EORDER[ei + 1]] = prefetch_disp(EORDER[ei + 1])

                    # ---------------- fc: h1 = gelu(x @ Wfc) ----------------
                    # CAP <= 512: single 512-wide segment (one PSUM bank).
                    # CAP > 512: split at 256 so both segments are >= 107ns
                    # of PE rows (no LDWEIGHTS stall on tiny tail matmuls).
                    # Segment list: (col0, width, h1 tile, tile stride)
                    h1all = h1tp.tile([P, KF * 512], BF16, tag="h1all",
                                      name=f"h1_{e}")
                    h1tl = None
                    if TAILW:
                        h1tl = h1tp.tile([P, KF * 384], BF16, tag="h1tl",
                                         name=f"h1t_{e}")
                        FSEG = [(0, 256, h1all, 512, ps_s1),
                                (256, CAP - 256, h1tl, 384, ps_st)]
                    else:
                        FSEG = [(0, CAP, h1all, 512, ps_s1)]
                    for ft in range(KF):
                        pss = []
                        for si, (_, _, _, _, pool) in enumerate(FSEG):
                            ptag = "ps1" if pool is ps_s1 else "pst"
                            pss.append(pool.tile([P, 512], F32, tag=ptag,
                                                 name=f"pss{si}"))
                        for k in range(KH):
                            for si, (c0, cwid, _, _, _) in enumerate(FSEG):
                                nc.tensor.matmul(
                                    out=pss[si][:, 0:cwid],
                                    lhsT=wfc_k[k][:, ft * P:(ft + 1) * P],
                                    rhs=xet3d[:, k * CAPR + c0:
                                              k * CAPR + c0 + cwid],
                                    start=(k == 0), stop=(k == KH - 1))
                        bias_ap = (zbias[:] if zero_bias
                                   else bfc_sb[:, ft:ft + 1])
                        for si, (c0, cwid, h1t_, hstride, _) in \
                                enumerate(FSEG):
                            nc.scalar.activation(
                                out=h1t_[:, ft * hstride:ft * hstride + cwid],
                                in_=pss[si][:, 0:cwid],
                                func=_ACT_FN, bias=bias_ap)

                    def h1_slice(k, cs, cw):
                        """lhsT slice for proj sub [cs, cs+cw)."""
                        if TAILW and cs >= 256:
                            return h1tl[:, k * 384 + cs - 256:
                                        k * 384 + cs - 256 + cw]
                        return h1all[:, k * 512 + cs:k * 512 + cs + cw]

                    # fc weights for e are no longer needed: stream e+2's now
                    # (scalar ring; WAR on the last fc matmul read above)
                    if ei + 2 < E:
                        wstage[EORDER[ei + 2]] = prefetch_weights(
                            EORDER[ei + 2], nc.scalar)

                    # ------------- proj: y = w * (h1 @ Wproj) ---------------
                    ysb = ysbp.tile([P, NSUB * H], F32, tag="ysb",
                                    name=f"ysb_{e}")
                    if CAP % P:
                        # zero the partial last sub's block: its rows cw:128
                        # are scatter-added (to the pad token) and must be 0
                        nc.vector.memset(
                            ysb[0:P, (NSUB - 1) * H:NSUB * H], 0.0)
                    for hi, (hs, hw) in enumerate(HT):
                        psy = [ps_y.tile([P, 512], F32, tag=f"psy{ci}",
                                         name=f"psy{ci}{hi}_{e}")
                               for ci in range(NSUBM)]
                        ptl = None
                        if TAILW:
                            ptl = ps_st.tile([P, 512], F32, tag="pst",
                                             name=f"ptl{hi}_{e}")
                        for k2 in range(KF // 2):
                            # load TWO k-chunks per DMA: halves the issue
                            # count so the ring keeps pace with the PE in
                            # the 256-wide pass
                            wp = wpp.tile([P, 2 * 512], BF16, tag="wp")
                            nc.sync.dma_start(
                                out=wp[:, :].rearrange(
                                    "p (b c) -> p b c", b=2)[:, :, 0:hw],
                                in_=wpj.ap()[e, 2 * k2 * P:
                                             (2 * k2 + 2) * P,
                                             hs:hs + hw].rearrange(
                                    "(b p) c -> p b c", p=P))
                            for b in range(2):
                                k = 2 * k2 + b
                                wps = wp[:, b * 512:b * 512 + hw]
                                for ci in range(NSUBM):
                                    cs, cw = SUBS[ci]
                                    nc.tensor.matmul(
                                        out=psy[ci][0:cw, 0:hw],
                                        lhsT=h1_slice(k, cs, cw),
                                        rhs=wps,
                                        start=(k == 0),
                                        stop=(zero_bias and k == KF - 1))
                                if TAILW:
                                    cs, cw = SUBS[4]
                                    nc.tensor.matmul(
                                        out=ptl[0:cw, 0:hw],
                                        lhsT=h1_slice(k, cs, cw),
                                        rhs=wps,
                                        start=(k == 0),
                                        stop=(zero_bias and k == KF - 1))
                        for ci in range(NSUB):
                            cs, cw = SUBS[ci]
                            pt = ptl if ci == 4 else psy[ci]
                            if not zero_bias:
                                nc.tensor.matmul(
                                    out=pt[0:cw, 0:hw],
                                    lhsT=ones_bf[:, 0:cw],
                                    rhs=bpj_sb[:, hs:hs + hw],
                                    start=False, stop=True)
                            nc.vector.tensor_scalar_mul(
                                ysb[0:cw, ci * H + hs:ci * H + hs + hw],
                                pt[0:cw, 0:hw],
                                w_t[ci][0:cw, 0:1])
                    # one scatter-add per expert: per-sub ucode calls
                    # serialize at ~5.7us each on the gpsimd queue
                    nc.gpsimd.dma_scatter_add(
                        out_ap=out.ap()[:, :],
                        in_ap=ysb[:, :].rearrange("p (n h) -> p n h", n=NSUB),
                        idxs_ap=idx[:, :], num_idxs=CAPR, num_idxs_reg=CAPR,
                        elem_size=H)

    nc.compile()
    return nc


# ---------------------------------------------------------------------------
_BUILD_CACHE = {}
_LAST_IN_MAPS = None


def _get_built(TLOC, H, F, E, CAPS, zero_bias=False):
    key = (TLOC, H, F, E, tuple(CAPS), zero_bias)
    if key not in _BUILD_CACHE:
        _BUILD_CACHE[key] = build_moe(TLOC, H, F, E, tuple(CAPS),
                                      zero_bias=zero_bias)
    return _BUILD_CACHE[key]


def kernel(hidden_states, Wr, br, Wfc, bfc, Wproj, bproj):
    from concourse.bass_utils import run_bass_kernel_spmd

    hs = np.ascontiguousarray(np.asarray(hidden_states, dtype=np.float32))
    Wr = np.ascontiguousarray(np.asarray(Wr, dtype=np.float32))
    br = np.ascontiguousarray(np.asarray(br, dtype=np.float32))
    Wfc = np.asarray(Wfc, dtype=np.float32)
    bfc = np.ascontiguousarray(np.asarray(bfc, dtype=np.float32))
    Wproj = np.asarray(Wproj, dtype=np.float32)
    bproj = np.asarray(bproj, dtype=np.float32)

    B, S, H = hs.shape
    E, H2, F = Wfc.shape
    assert H2 == H
    T = B * S
    assert T % N_CORES == 0
    TLOC = T // N_CORES
    x = hs.reshape(T, H)

    # host-side routing peek ONLY to pick compile-time capacities and the
    # balanced token->core assignment (routing itself runs on device)
    logits = x @ Wr + br[None, :]
    top2 = np.argpartition(-logits, 2, axis=1)[:, :2]
    pair_lo = np.minimum(top2[:, 0], top2[:, 1])
    pair_hi = np.maximum(top2[:, 0], top2[:, 1])
    type_id = pair_lo.astype(np.int64) * E + pair_hi
    order = np.argsort(type_id, kind="stable")
    # deal the type-sorted token list round-robin: core g takes order[g::8]
    core_toks = [order[g::N_CORES] for g in range(N_CORES)]
    maxcnt = np.zeros(E, dtype=np.int64)
    for g in range(N_CORES):
        cnts = np.bincount(top2[core_toks[g]].ravel(), minlength=E)
        maxcnt = np.maximum(maxcnt, cnts)
    CAPS = tuple(int(max(128, math.ceil((m + 4) / 8.0) * 8))
                 for m in maxcnt)

    zero_bias = bool(np.all(bfc == 0.0) and np.all(bproj == 0.0)
                     and np.all(br == 0.0))
    nc = _get_built(TLOC, H, F, E, CAPS, zero_bias)

    wfc_bf = np.ascontiguousarray(Wfc.astype(ml_dtypes.bfloat16))
    wpj_bf = np.ascontiguousarray(Wproj.astype(ml_dtypes.bfloat16))

    in_maps = []
    for g in range(N_CORES):
        xc = x[core_toks[g]]
        xbf_c = np.zeros((TLOC + 128, H), dtype=ml_dtypes.bfloat16)
        xbf_c[:TLOC] = xc.astype(ml_dtypes.bfloat16)
        m = {
            "xt": np.ascontiguousarray(xc.T),
            "xbf": xbf_c,
            "wr": Wr,
            "wfc": wfc_bf,
            "wpj": wpj_bf,
        }
        if not zero_bias:
            m["brr"] = np.ascontiguousarray(br.reshape(1, E))
            m["bfc"] = bfc
            m["bpj"] = np.ascontiguousarray(bproj.astype(ml_dtypes.bfloat16))
        in_maps.append(m)

    global _LAST_IN_MAPS
    _LAST_IN_MAPS = in_maps

    res = run_bass_kernel_spmd(nc, in_maps, core_ids=list(range(N_CORES)))
    full = np.empty((T, H), dtype=np.float32)
    for g in range(N_CORES):
        full[core_toks[g]] = res.results[g]["out"][:TLOC]
    return full.reshape(B, S, H).astype(np.float32)
